# revision 18
# baseline (speedup 1.0000x reference)
"""SwiGLU-projected causal MHA (B=4, S=2048, D=1024, H=16) on 8 TRN2 NeuronCores.

Sharding: core c -> (batch b = c//2, head-group g = c%2).  Each core computes
the SwiGLU Q/K/V projections for its 512 output channels (= 8 heads) of its
batch, runs causal attention for those heads, and produces a partial output
projection (contraction over its 512 channels).  The host sums the two
partials per batch and adds the output bias.

v2: software-pipelined emission.  Projections are split into fine-grained
items; attention for q-group t starts right after q(t)/k(t) are projected,
and the remaining projection items (v(t), q(t+1), k(t+1)) are dripped into
the attention kc loops to fill the PE idle created by the ACT-bound exp
stream.  Score matmul pairs use explicit row-group tile_position for PE
concurrency; exp runs once per kc over both heads of a pair via a strided
2-bank PSUM read; diagonal blocks are column-truncated (causal); the partial
mask multiply runs on the Pool engine; softmax normalization uses
reciprocal_approx_fast plus a single selector matmul to broadcast both
reciprocal rows in one shot.
"""
import sys

sys.path.insert(0, "/opt/trn_rl_repo")
import numpy as np

import concourse.bacc as bacc
import concourse.tile as tile
import concourse.mybir as mybir

B, S, D = 4, 2048, 1024
H, DK = 16, 64
NCORES = 8
GCH = 512          # channels per core (8 heads)
NT = S // 128      # 16 seq chunks
F32 = mybir.dt.float32
F32R = mybir.dt.float32r
BF16 = mybir.dt.bfloat16
ACTF = mybir.ActivationFunctionType
ALU = mybir.AluOpType

TRACE = False          # set by test.py for profiling runs
TRACE_CORES = None
LAST_RESULT = None     # BassKernelResults stash for test.py

DRIP_EVERY = 6         # attention kc-units between dripped projection items
SCORE_TILE_POS = True  # explicit row-group tile_position on score MM pairs


def build_program(mask_mode):
    """mask_mode: 'causal' (tril), 'full' (all ones), 'general' (arbitrary)."""
    nc = bacc.Bacc("TRN2", target_bir_lowering=False, debug=False)

    xT = {s: nc.dram_tensor(f"x{s}T", [128, 4, 8, 512], BF16,
                            kind="ExternalInput")
          for s in "qkv"}
    w1T = {s: nc.dram_tensor(f"w1T_{s}", [128, 8, GCH], BF16,
                             kind="ExternalInput")
           for s in "qkv"}
    w2T = {s: nc.dram_tensor(f"w2T_{s}", [128, 8, GCH], BF16,
                             kind="ExternalInput")
           for s in "qkv"}
    bias_d = {}
    for s in "qk":
        for bn in ("b1", "b2", "b1h"):
            bias_d[f"{bn}_{s}"] = nc.dram_tensor(f"{bn}_{s}", [128, 4], F32,
                                                 kind="ExternalInput")
    b1v_d = nc.dram_tensor("b1_v", [1, GCH], BF16, kind="ExternalInput")
    b2v_d = nc.dram_tensor("b2_v", [1, GCH], BF16, kind="ExternalInput")
    woT_d = nc.dram_tensor("woT", [128, 4, D], BF16, kind="ExternalInput")
    patc_d = neye_d = m01T_d = None
    if mask_mode == "causal":
        patc_d = nc.dram_tensor("patc", [128, 128], BF16, kind="ExternalInput")
        neye_d = nc.dram_tensor("neye", [128, 128], BF16, kind="ExternalInput")
    elif mask_mode == "general":
        m01T_d = nc.dram_tensor("m01T", [S, S], BF16, kind="ExternalInput")
    pout_d = nc.dram_tensor("pout", [S, D], F32, kind="ExternalOutput")

    def kc_count(qg):
        return 4 * qg + 4 if mask_mode == "causal" else NT

    with tile.TileContext(nc) as tc:
        with (
            tc.tile_pool(name="persist", bufs=1) as persist,
            tc.tile_pool(name="xq", bufs=2) as xq_pool,
            tc.tile_pool(name="xk", bufs=2) as xk_pool,
            tc.tile_pool(name="xv", bufs=2) as xv_pool,
            tc.tile_pool(name="stage", bufs=3) as stage,
            tc.tile_pool(name="attnp", bufs=3) as attnp,
            tc.tile_pool(name="ctp", bufs=2) as ctp,
            tc.tile_pool(name="smalls", bufs=2) as smalls,
            tc.tile_pool(name="ostage", bufs=2) as ostage,
            tc.tile_pool(name="mpool", bufs=2) as mpool,
            tc.tile_pool(name="scps", bufs=2, space="PSUM") as scps,
            tc.tile_pool(name="cxps", bufs=2, space="PSUM") as cxps,
        ):
            xpools = {"q": xq_pool, "k": xk_pool, "v": xv_pool}

            # ---------------- persistent tiles + upfront DMAs ------------
            # DMA emission order tracks first use: x(q,0) + w_q first.
            qt_sb = persist.tile([128, 4, S], BF16, tag="qt")
            kt_sb = persist.tile([128, 4, S], BF16, tag="kt")
            v_sb = persist.tile([128, NT, 8, 65], BF16, tag="v")

            x_tiles = {}

            def fetch_x(s, t):
                if (s, t) in x_tiles:
                    return x_tiles[(s, t)]
                xt = xpools[s].tile([128, 8, 512], BF16, tag=f"x{s}")
                for h in range(2):
                    nc.sync.dma_start(
                        xt[:, 4 * h:4 * h + 4, :],
                        xT[s][:, t, 4 * h:4 * h + 4, :],
                    )
                x_tiles[(s, t)] = xt
                return xt

            w_sb = {}

            def fetch_w(s):
                w1sb = persist.tile([128, 8, GCH], BF16, tag=f"w1{s}")
                w2sb = persist.tile([128, 8, GCH], BF16, tag=f"w2{s}")
                for wsb, wd in ((w1sb, w1T[s]), (w2sb, w2T[s])):
                    for h in range(2):
                        nc.sync.dma_start(
                            wsb[:, 4 * h:4 * h + 4, :],
                            wd[:, 4 * h:4 * h + 4, :])
                w_sb[s] = (w1sb, w2sb)

            fetch_x("q", 0)
            fetch_w("q")
            fetch_x("k", 0)
            fetch_w("k")
            fetch_x("v", 0)
            fetch_w("v")

            bias_sb = {}
            for s in "qk":
                for bn in ("b1", "b2", "b1h"):
                    t_ = persist.tile([128, 4], F32, tag=f"{bn}{s}")
                    nc.sync.dma_start(t_[:], bias_d[f"{bn}_{s}"][:])
                    bias_sb[f"{bn}_{s}"] = t_
            b1vr = persist.tile([1, GCH], BF16, tag="b1v")
            b2vr = persist.tile([1, GCH], BF16, tag="b2v")
            nc.sync.dma_start(b1vr[:], b1v_d[:])
            nc.sync.dma_start(b2vr[:], b2v_d[:])
            woT_sb = persist.tile([128, 4, D], BF16, tag="wo")
            nc.sync.dma_start(woT_sb[:], woT_d[:])

            onesf = persist.tile([1, 128], F32, tag="onesf")
            ones_r = persist.tile([1, 128], BF16, tag="ones_r")
            nc.any.memset(onesf[:], 1.0)
            nc.vector.tensor_copy(ones_r[:], onesf[:])
            onescol = persist.tile([128, 1], F32, tag="onescol")
            nc.any.memset(onescol[:], 1.0)
            nc.vector.tensor_copy(
                v_sb[:, :, :, 64:65],
                onescol[:, None, :].to_broadcast([128, NT, 8, 1]),
            )
            # selector for the denominator broadcast: one K=33 matmul puts
            # rec row 0 on out rows 0..63 and rec row 32 on rows 64..127.
            # (partition bases must be 32-aligned, hence rows 0/32; the
            # pad rows stay memset so 0*pad contributes exact zeros.)
            sel2 = persist.tile([33, 128], BF16, tag="sel2")
            nc.any.memset(sel2[:], 0.0)
            nc.any.memset(sel2[0:1, 0:64], 1.0)
            nc.any.memset(sel2[32:33, 64:128], 1.0)
            den_sb = persist.tile([33, 512], F32, tag="den")
            nc.any.memset(den_sb[:], 1.0)
            rec_sb = persist.tile([33, 512], F32, tag="rec")
            nc.any.memset(rec_sb[:], 1.0)
            rec_bf = persist.tile([33, 512], BF16, tag="recbf")
            nc.any.memset(rec_bf[:], 1.0)

            patc_sb = neye_sb = None
            if mask_mode == "causal":
                patc_sb = persist.tile([128, 128], BF16, tag="patc")
                nc.sync.dma_start(patc_sb[:], patc_d[:])
                neye_sb = persist.tile([128, 128], BF16, tag="neye")
                nc.sync.dma_start(neye_sb[:], neye_d[:])

            # ------------------- projection items ------------------------
            def emit_qk_item(s, t, jh, jj):
                """16 MMs + epilogue for channel chunk j=jh*2+jj, seq tile t."""
                j = jh * 2 + jj
                xt = x_tiles[(s, t)]
                w1sb, w2sb = w_sb[s]
                pp = scps.tile([128, 2, 512], F32, tag="sc",
                               name=f"pp{s}{t}{j}")
                ps1, ps2 = pp[:, 0, :], pp[:, 1, :]
                jsl = slice(j * 128, (j + 1) * 128)
                for dc in range(8):
                    nc.tensor.matmul(ps1, w1sb[:, dc, jsl], xt[:, dc, :],
                                     start=(dc == 0), stop=(dc == 7))
                    nc.tensor.matmul(ps2, w2sb[:, dc, jsl], xt[:, dc, :],
                                     start=(dc == 0), stop=(dc == 7))
                act = stage.tile([128, 512], F32, tag="act")
                # act = tanh((ps1+b1)/2)
                nc.scalar.activation(act[:], ps1, ACTF.Tanh, scale=0.5,
                                     bias=bias_sb[f"b1h_{s}"][:, j:j + 1])
                a_sb = stage.tile([128, 512], F32, tag="asb")
                nc.vector.tensor_scalar_add(a_sb[:], ps1,
                                            bias_sb[f"b1_{s}"][:, j:j + 1])
                # act = (1+tanh)*(ps1+b1) = 2*silu(ps1+b1)
                nc.vector.scalar_tensor_tensor(
                    act[:], act[:], 1.0, a_sb[:], op0=ALU.add, op1=ALU.mult)
                dst = (qt_sb if s == "q" else kt_sb)[
                    :, j, t * 512:(t + 1) * 512]
                nc.vector.scalar_tensor_tensor(
                    dst, ps2, bias_sb[f"b2_{s}"][:, j:j + 1], act[:],
                    op0=ALU.add, op1=ALU.mult)

            def emit_v_item(t, j):
                """16+2 MMs + epilogue for seq block nt=t*4+j (v path)."""
                xt = x_tiles[("v", t)]
                w1sb, w2sb = w_sb["v"]
                pp = scps.tile([128, 2, 512], F32, tag="sc",
                               name=f"ppv{t}{j}")
                ps1, ps2 = pp[:, 0, :], pp[:, 1, :]
                jsl = slice(j * 128, (j + 1) * 128)
                for dc in range(8):
                    nc.tensor.matmul(ps1, xt[:, dc, jsl], w1sb[:, dc, :],
                                     start=(dc == 0), stop=False)
                    nc.tensor.matmul(ps2, xt[:, dc, jsl], w2sb[:, dc, :],
                                     start=(dc == 0), stop=False)
                nc.tensor.matmul(ps1, ones_r[:], b1vr[:],
                                 start=False, stop=True)
                nc.tensor.matmul(ps2, ones_r[:], b2vr[:],
                                 start=False, stop=True)
                act = stage.tile([128, 512], F32, tag="act")
                nc.scalar.activation(act[:], ps1, ACTF.Tanh, scale=0.5)
                # act = (1+tanh)*ps1 = 2*silu(ps1)
                nc.vector.scalar_tensor_tensor(
                    act[:], act[:], 1.0, ps1, op0=ALU.add, op1=ALU.mult)
                nt_i = t * 4 + j
                nc.vector.tensor_tensor(
                    v_sb[:, nt_i, :, 0:64],
                    ps2.rearrange("p (h d) -> p h d", h=8),
                    act[:].rearrange("p (h d) -> p h d", h=8),
                    ALU.mult)

            # ------------------- attention helpers ------------------------
            def emit_oproj_chunk(qg, ns, oh, ct_qg):
                nt_i = qg * 4 + ns
                nsl = slice(ns * 128, (ns + 1) * 128)
                pof = scps.tile([128, 2, 512], F32, tag="sc",
                                name=f"po{qg}{ns}{oh}")
                po = pof[:, 0, :]
                for j in range(4):
                    nc.tensor.matmul(
                        po, ct_qg[:, j, nsl],
                        woT_sb[:, j, oh * 512:(oh + 1) * 512],
                        start=(j == 0), stop=(j == 3))
                ot = ostage.tile([128, 512], F32, tag="ot")
                nc.vector.tensor_copy(ot[:], po)
                nc.sync.dma_start(
                    pout_d[nt_i * 128:(nt_i + 1) * 128,
                           oh * 512:(oh + 1) * 512],
                    ot[:])

            # ===================== main pipeline ==========================
            # q(0), k(0) projections up front (x DMAs pipelined by pools).
            for s in ("q", "k"):
                for jh in range(2):
                    for jj in range(2):
                        emit_qk_item(s, 0, jh, jj)
            if mask_mode != "causal":
                # attention for every q-group needs ALL of k and v: emit
                # the remaining projections up front, drip only q(t>0).
                for t in range(1, 4):
                    fetch_x("k", t)
                    for jh in range(2):
                        for jj in range(2):
                            emit_qk_item("k", t, jh, jj)
                for t in range(4):
                    fetch_x("v", t)
                    for j in range(4):
                        emit_v_item(t, j)

            pending = []          # deferred projection items (drip queue)
            oproj_pending = []    # deferred oproj chunks of previous qg
            norm_pending = []     # deferred normalization part-B closures

            def drip(n=1):
                for _ in range(n):
                    if pending:
                        pending.pop(0)()
                    elif oproj_pending:
                        oproj_pending.pop(0)()

            for qg in range(4):
                kcmax = kc_count(qg)
                qsl0 = qg * 512
                ct_qg = ctp.tile([128, 4, 512], BF16, tag="ct",
                                 name=f"ct{qg}")

                # stock the drip queue with next tile's q/k items; v(qg)
                # items are emitted in-line right before their first use.
                if qg < 3:
                    fetch_x("q", qg + 1)
                    if mask_mode == "causal":
                        fetch_x("k", qg + 1)
                        fetch_x("v", qg + 1)
                    srcs = ("q", "k") if mask_mode == "causal" else ("q",)
                    for s in srcs:
                        for jh in range(2):
                            for jj in range(2):
                                pending.append(
                                    (lambda s=s, t=qg + 1, jh=jh, jj=jj:
                                     emit_qk_item(s, t, jh, jj)))

                mtiles = None
                if mask_mode == "general":
                    mtiles = []
                    mt_sb = mpool.tile([128, NT, 512], BF16, tag="mt")
                    for kc in range(kcmax):
                        nc.sync.dma_start(
                            mt_sb[:, kc, :],
                            m01T_d[kc * 128:(kc + 1) * 128,
                                   qsl0:qsl0 + 512])
                        mtiles.append(mt_sb[:, kc, :])

                units = 0
                for pj in range(4):
                    ctx = cxps.tile([128, 2, 512], F32, tag="cx",
                                    name=f"ctx{qg}{pj}")
                    for kc in range(kcmax):
                        ksl = slice(kc * 128, (kc + 1) * 128)
                        diag_i = kc - 4 * qg
                        c0 = 128 * diag_i if (
                            mask_mode == "causal" and diag_i >= 0) else 0
                        ncols = 512 - c0
                        qca = slice(qsl0 + c0, qsl0 + 512)

                        diag = mask_mode == "causal" and diag_i >= 0
                        sc = scps.tile([128, 2, 512], F32, tag="sc",
                                       name=f"sc{qg}{pj}{kc}")
                        for par in range(2):
                            bp = par * 64
                            tp = (bp, 0) if SCORE_TILE_POS else None
                            nc.tensor.matmul(
                                sc[:, par, c0:],
                                kt_sb[bp:bp + 64, pj, ksl],
                                qt_sb[bp:bp + 64, pj, qca],
                                start=True, stop=not diag,
                                tile_position=tp)
                        if diag:
                            # add -1e9 to the masked (upper) triangle of the
                            # partial block so exp() yields exact zeros.
                            for par in range(2):
                                nc.tensor.matmul(
                                    sc[:, par, c0:c0 + 128],
                                    neye_sb[:], patc_sb[:],
                                    start=False, stop=True)

                        # v(qg) projection items drip in right after the
                        # first pj's scores that need them.
                        if pj == 0 and mask_mode == "causal" and diag_i >= 0:
                            emit_v_item(qg, diag_i)

                        attn = attnp.tile([128, 2, 512], BF16, tag="at",
                                          name=f"at{qg}{pj}{kc}")
                        nc.scalar.activation(attn[:, 0, c0:], sc[:, 0, c0:],
                                             ACTF.Exp)
                        nc.scalar.activation(attn[:, 1, c0:], sc[:, 1, c0:],
                                             ACTF.Exp)
                        if mask_mode == "general":
                            nc.gpsimd.tensor_tensor(
                                attn[:], attn[:],
                                mtiles[kc][:, None, :].to_broadcast(
                                    [128, 2, 512]),
                                ALU.mult)
                        for par in range(2):
                            hl = 2 * pj + par
                            nc.tensor.matmul(
                                ctx[0:65, par, c0:],
                                v_sb[:, kc, hl, :],
                                attn[:, par, c0:],
                                start=(kc == 0),
                                stop=(kc == kcmax - 1))
                        units += 1
                        if kc == 2 and norm_pending:
                            norm_pending.pop(0)()
                        if units % DRIP_EVERY == 0:
                            drip()

                    # fill the PE queue before the serial norm chain:
                    # oproj chunks of the previous q-group + one drip item.
                    for _ in range(2):
                        if oproj_pending:
                            oproj_pending.pop(0)()
                    if pending:
                        pending.pop(0)()

                    # ---- normalization, part A: DVE reciprocal chain ----
                    nc.vector.tensor_copy(den_sb[0:1, :], ctx[64:65, 0, :])
                    nc.vector.tensor_copy(den_sb[32:33, :], ctx[64:65, 1, :])
                    nc.vector.reciprocal_approx_fast(rec_sb[:], den_sb[:])
                    nc.vector.tensor_copy(rec_bf[:], rec_sb[:])

                    # part B (bc matmul + ct multiplies) is deferred into the
                    # next head pair's kc loop so the PE FIFO never waits on
                    # the DVE chain above.
                    def emit_normB(qg=qg, pj=pj, ctx=ctx, ct_qg=ct_qg):
                        bc_full = scps.tile([128, 2, 512], F32, tag="sc",
                                            name=f"bc{qg}{pj}")
                        bc_ps = bc_full[:, 0, :]
                        nc.tensor.matmul(bc_ps, sel2[:], rec_bf[:])
                        bc_sb = smalls.tile([128, 512], BF16, tag="bcs")
                        nc.vector.tensor_copy(bc_sb[:], bc_ps)
                        for par in range(2):
                            bp = par * 64
                            nc.vector.tensor_tensor(
                                ct_qg[bp:bp + 64, pj, :],
                                ctx[0:64, par, :],
                                bc_sb[bp:bp + 64, :], ALU.mult)
                    norm_pending.append(emit_normB)

                # flush remaining projection items before the next q-group
                while pending:
                    drip()
                while norm_pending:
                    norm_pending.pop(0)()
                while oproj_pending:
                    oproj_pending.pop(0)()
                for ns in range(4):
                    for oh in range(2):
                        oproj_pending.append(
                            (lambda qg=qg, ns=ns, oh=oh, ct=ct_qg:
                             emit_oproj_chunk(qg, ns, oh, ct)))

            while oproj_pending:
                oproj_pending.pop(0)()
    nc.compile()
    return nc


def _host_prepare(inputs):
    """Split the full problem into 8 per-core input maps + host-side info."""
    q = np.asarray(inputs["query"], dtype=np.float32)
    k = np.asarray(inputs["key"], dtype=np.float32)
    v = np.asarray(inputs["value"], dtype=np.float32)
    mask = np.asarray(inputs["mask"])
    w = {n: np.asarray(inputs[n], dtype=np.float32)
         for n in ("wq1", "wq2", "wk1", "wk2", "wv1", "wv2", "wo")}
    bias = {n: np.asarray(inputs[n], dtype=np.float32)
            for n in ("bq1", "bq2", "bk1", "bk2", "bv1", "bv2", "bo")}

    m = mask.reshape(S, S)
    if np.array_equal(m != 0, np.tril(np.ones((S, S), bool))):
        mask_mode = "causal"
    elif np.all(m != 0):
        mask_mode = "full"
    else:
        mask_mode = "general"

    patc = None
    m01T = None
    if mask_mode == "causal":
        kk = np.arange(128)[:, None]
        qq = np.arange(128)[None, :]
        patc = np.ascontiguousarray((kk > qq).astype(np.float32))
    elif mask_mode == "general":
        m01T = np.ascontiguousarray((m != 0).T.astype(np.float32))

    scale = 1.0 / np.sqrt(DK).astype(np.float32)

    import ml_dtypes

    def cvt(a):
        return np.ascontiguousarray(a).astype(ml_dtypes.bfloat16)

    in_maps = []
    for c in range(NCORES):
        b, g = divmod(c, 2)
        sl = slice(g * GCH, (g + 1) * GCH)
        def xprep(a):
            # [D, S] -> [128 p, 4 t, 8 dc, 512 o]; x[dc*128+p, t*512+o]
            return cvt(a.T.reshape(8, 128, 4, 512).transpose(1, 2, 0, 3))

        def wprep(a):
            # [D, GCH] -> [128 p, 8 dc, 512 o]; w[dc*128+p, o]
            return cvt(a.reshape(8, 128, GCH).transpose(1, 0, 2))

        im = {
            "xqT": xprep(q[b]),
            "xkT": xprep(k[b]),
            "xvT": xprep(v[b]),
            "w1T_q": wprep(w["wq1"][sl].T),
            # fold the 1/sqrt(dk) score scale into the non-silu Q branch,
            # and 0.5 everywhere (silu computed as A*(1+tanh(A/2)) = 2*silu)
            "w2T_q": wprep(w["wq2"][sl].T * (scale * 0.5)),
            "w2T_k": wprep(w["wk2"][sl].T * 0.5),
            "w2T_v": wprep(w["wv2"][sl].T * 0.5),
            "w1T_k": wprep(w["wk1"][sl].T),
            "w1T_v": wprep(w["wv1"][sl].T),
            "b1_q": np.ascontiguousarray(bias["bq1"][sl].reshape(4, 128).T),
            "b1h_q": np.ascontiguousarray(
                (bias["bq1"][sl] * 0.5).reshape(4, 128).T),
            "b2_q": np.ascontiguousarray(
                (bias["bq2"][sl] * (scale * 0.5)).reshape(4, 128).T),
            "b1_k": np.ascontiguousarray(bias["bk1"][sl].reshape(4, 128).T),
            "b1h_k": np.ascontiguousarray(
                (bias["bk1"][sl] * 0.5).reshape(4, 128).T),
            "b2_k": np.ascontiguousarray(
                (bias["bk2"][sl] * 0.5).reshape(4, 128).T),
            "b1_v": cvt(bias["bv1"][sl].reshape(1, GCH)),
            "b2_v": cvt((bias["bv2"][sl] * 0.5).reshape(1, GCH)),
            "woT": cvt(
                w["wo"][:, sl].T.reshape(4, 128, D).transpose(1, 0, 2)),
        }
        if mask_mode == "causal":
            im["patc"] = cvt(patc)
            im["neye"] = cvt(-1e9 * np.eye(128, dtype=np.float32))
        elif mask_mode == "general":
            im["m01T"] = cvt(m01T)
        in_maps.append(im)
    return mask_mode, in_maps, bias["bo"]


def kernel(**inputs):
    global LAST_RESULT
    mask_mode, in_maps, bo = _host_prepare(inputs)
    nc = build_program(mask_mode)

    import concourse.bass_utils as bu

    if TRACE:
        import types

        try:
            from trn_agent_boot.trn_boot import _ntff_profile_via_ctypes

            hook = _ntff_profile_via_ctypes("/opt/axon/libaxon_pjrt.so")
            m = types.ModuleType("antenv.axon_hooks")
            m.get_axon_ntff_profile_hook = lambda: hook
            import antenv  # noqa: F401

            sys.modules["antenv.axon_hooks"] = m
            bu.upload_artifacts = lambda d: "local://skipped"
        except Exception as e:
            print("profiling hook install failed:", e)

    res = bu.run_bass_kernel_spmd(
        nc, in_maps, core_ids=list(range(NCORES)),
        trace=TRACE, trace_cores=TRACE_CORES,
    )
    LAST_RESULT = res

    out = np.empty((B, S, D), dtype=np.float32)
    for b in range(B):
        out[b] = (res.results[2 * b]["pout"] + res.results[2 * b + 1]["pout"]
                  + bo[None, :])
    return out


# revision 19
# speedup vs baseline: 1.1345x; 1.1345x over previous
"""SwiGLU-projected causal MHA (B=4, S=2048, D=1024, H=16) on 8 TRN2 NeuronCores.

Sharding: core c -> (batch b = c//2, head-group g = c%2).  Each core computes
the SwiGLU Q/K/V projections for its 512 output channels (= 8 heads) of its
batch, runs causal attention for those heads, and produces a partial output
projection (contraction over its 512 channels).  The host sums the two
partials per batch and adds the output bias.

v2: software-pipelined emission.  Projections are split into fine-grained
items; attention for q-group t starts right after q(t)/k(t) are projected,
and the remaining projection items (v(t), q(t+1), k(t+1)) are dripped into
the attention kc loops to fill the PE idle created by the ACT-bound exp
stream.  Score matmul pairs use explicit row-group tile_position for PE
concurrency; exp runs once per kc over both heads of a pair via a strided
2-bank PSUM read; diagonal blocks are column-truncated (causal); the partial
mask multiply runs on the Pool engine; softmax normalization uses
reciprocal_approx_fast plus a single selector matmul to broadcast both
reciprocal rows in one shot.
"""
import sys

sys.path.insert(0, "/opt/trn_rl_repo")
import numpy as np

import concourse.bacc as bacc
import concourse.tile as tile
import concourse.mybir as mybir

B, S, D = 4, 2048, 1024
H, DK = 16, 64
NCORES = 8
GCH = 512          # channels per core (8 heads)
NT = S // 128      # 16 seq chunks
F32 = mybir.dt.float32
F32R = mybir.dt.float32r
BF16 = mybir.dt.bfloat16
ACTF = mybir.ActivationFunctionType
ALU = mybir.AluOpType

TRACE = False          # set by test.py for profiling runs
TRACE_CORES = None
LAST_RESULT = None     # BassKernelResults stash for test.py

DRIP_EVERY = 4         # attention kc-units between dripped projection items
SCORE_TILE_POS = True  # explicit row-group tile_position on score MM pairs


def build_program(mask_mode):
    """mask_mode: 'causal' (tril), 'full' (all ones), 'general' (arbitrary)."""
    nc = bacc.Bacc("TRN2", target_bir_lowering=False, debug=False)

    xT = {s: nc.dram_tensor(f"x{s}T", [128, 4, 8, 512], BF16,
                            kind="ExternalInput")
          for s in "qkv"}
    w1T = {s: nc.dram_tensor(f"w1T_{s}", [128, 8, GCH], BF16,
                             kind="ExternalInput")
           for s in "qkv"}
    w2T = {s: nc.dram_tensor(f"w2T_{s}", [128, 8, GCH], BF16,
                             kind="ExternalInput")
           for s in "qkv"}
    bias_d = {}
    for s in "qk":
        for bn in ("b1", "b2", "b1h"):
            bias_d[f"{bn}_{s}"] = nc.dram_tensor(f"{bn}_{s}", [128, 4], F32,
                                                 kind="ExternalInput")
    b1v_d = nc.dram_tensor("b1_v", [1, GCH], BF16, kind="ExternalInput")
    b2v_d = nc.dram_tensor("b2_v", [1, GCH], BF16, kind="ExternalInput")
    woT_d = nc.dram_tensor("woT", [128, 4, D], BF16, kind="ExternalInput")
    patc_d = neye_d = m01T_d = None
    if mask_mode == "causal":
        patc_d = nc.dram_tensor("patc", [128, 128], BF16, kind="ExternalInput")
        neye_d = nc.dram_tensor("neye", [128, 128], BF16, kind="ExternalInput")
    elif mask_mode == "general":
        m01T_d = nc.dram_tensor("m01T", [S, S], BF16, kind="ExternalInput")
    pout_d = nc.dram_tensor("pout", [S, D], F32, kind="ExternalOutput")

    def kc_count(qg):
        return 4 * qg + 4 if mask_mode == "causal" else NT

    with tile.TileContext(nc) as tc:
        with (
            tc.tile_pool(name="persist", bufs=1) as persist,
            tc.tile_pool(name="xq", bufs=2) as xq_pool,
            tc.tile_pool(name="xk", bufs=2) as xk_pool,
            tc.tile_pool(name="xv", bufs=2) as xv_pool,
            tc.tile_pool(name="stage", bufs=3) as stage,
            tc.tile_pool(name="attnp", bufs=3) as attnp,
            tc.tile_pool(name="ctp", bufs=2) as ctp,
            tc.tile_pool(name="smalls", bufs=2) as smalls,
            tc.tile_pool(name="ostage", bufs=2) as ostage,
            tc.tile_pool(name="mpool", bufs=2) as mpool,
            tc.tile_pool(name="scps", bufs=2, space="PSUM") as scps,
            tc.tile_pool(name="cxps", bufs=2, space="PSUM") as cxps,
        ):
            xpools = {"q": xq_pool, "k": xk_pool, "v": xv_pool}

            # ---------------- persistent tiles + upfront DMAs ------------
            # DMA emission order tracks first use: x(q,0) + w_q first.
            qt_sb = persist.tile([128, 4, S], BF16, tag="qt")
            kt_sb = persist.tile([128, 4, S], BF16, tag="kt")
            v_sb = persist.tile([128, NT, 8, 65], BF16, tag="v")

            x_tiles = {}

            def fetch_x(s, t):
                if (s, t) in x_tiles:
                    return x_tiles[(s, t)]
                xt = xpools[s].tile([128, 8, 512], BF16, tag=f"x{s}")
                for h in range(2):
                    nc.sync.dma_start(
                        xt[:, 4 * h:4 * h + 4, :],
                        xT[s][:, t, 4 * h:4 * h + 4, :],
                    )
                x_tiles[(s, t)] = xt
                return xt

            w_sb = {}

            def fetch_w(s):
                w1sb = persist.tile([128, 8, GCH], BF16, tag=f"w1{s}")
                w2sb = persist.tile([128, 8, GCH], BF16, tag=f"w2{s}")
                for wsb, wd in ((w1sb, w1T[s]), (w2sb, w2T[s])):
                    for h in range(2):
                        nc.sync.dma_start(
                            wsb[:, 4 * h:4 * h + 4, :],
                            wd[:, 4 * h:4 * h + 4, :])
                w_sb[s] = (w1sb, w2sb)

            fetch_x("q", 0)
            fetch_w("q")
            fetch_x("k", 0)
            fetch_w("k")
            fetch_x("v", 0)
            fetch_w("v")

            bias_sb = {}
            for s in "qk":
                for bn in ("b1", "b2", "b1h"):
                    t_ = persist.tile([128, 4], F32, tag=f"{bn}{s}")
                    nc.sync.dma_start(t_[:], bias_d[f"{bn}_{s}"][:])
                    bias_sb[f"{bn}_{s}"] = t_
            b1vr = persist.tile([1, GCH], BF16, tag="b1v")
            b2vr = persist.tile([1, GCH], BF16, tag="b2v")
            nc.sync.dma_start(b1vr[:], b1v_d[:])
            nc.sync.dma_start(b2vr[:], b2v_d[:])
            woT_sb = persist.tile([128, 4, D], BF16, tag="wo")
            nc.sync.dma_start(woT_sb[:], woT_d[:])

            onesf = persist.tile([1, 128], F32, tag="onesf")
            ones_r = persist.tile([1, 128], BF16, tag="ones_r")
            nc.any.memset(onesf[:], 1.0)
            nc.vector.tensor_copy(ones_r[:], onesf[:])
            onescol = persist.tile([128, 1], F32, tag="onescol")
            nc.any.memset(onescol[:], 1.0)
            nc.vector.tensor_copy(
                v_sb[:, :, :, 64:65],
                onescol[:, None, :].to_broadcast([128, NT, 8, 1]),
            )
            # selector for the denominator broadcast: one K=33 matmul puts
            # rec row 0 on out rows 0..63 and rec row 32 on rows 64..127.
            # (partition bases must be 32-aligned, hence rows 0/32; the
            # pad rows stay memset so 0*pad contributes exact zeros.)
            sel2 = persist.tile([33, 128], BF16, tag="sel2")
            nc.any.memset(sel2[:], 0.0)
            nc.any.memset(sel2[0:1, 0:64], 1.0)
            nc.any.memset(sel2[32:33, 64:128], 1.0)
            den_sb = persist.tile([33, 512], F32, tag="den")
            nc.any.memset(den_sb[:], 1.0)
            rec_sb = persist.tile([33, 512], F32, tag="rec")
            nc.any.memset(rec_sb[:], 1.0)
            rec_bf = persist.tile([33, 512], BF16, tag="recbf")
            nc.any.memset(rec_bf[:], 1.0)

            patc_sb = neye_sb = None
            if mask_mode == "causal":
                patc_sb = persist.tile([128, 128], BF16, tag="patc")
                nc.sync.dma_start(patc_sb[:], patc_d[:])
                neye_sb = persist.tile([128, 128], BF16, tag="neye")
                nc.sync.dma_start(neye_sb[:], neye_d[:])

            # ------------------- projection items ------------------------
            def emit_qk_item(s, t, jh, jj):
                """16 MMs + epilogue for channel chunk j=jh*2+jj, seq tile t."""
                j = jh * 2 + jj
                xt = x_tiles[(s, t)]
                w1sb, w2sb = w_sb[s]
                pp = scps.tile([128, 2, 512], F32, tag="sc",
                               name=f"pp{s}{t}{j}")
                ps1, ps2 = pp[:, 0, :], pp[:, 1, :]
                jsl = slice(j * 128, (j + 1) * 128)
                for dc in range(8):
                    nc.tensor.matmul(ps1, w1sb[:, dc, jsl], xt[:, dc, :],
                                     start=(dc == 0), stop=(dc == 7))
                    nc.tensor.matmul(ps2, w2sb[:, dc, jsl], xt[:, dc, :],
                                     start=(dc == 0), stop=(dc == 7))
                act = stage.tile([128, 512], F32, tag="act")
                # act = tanh((ps1+b1)/2)
                nc.scalar.activation(act[:], ps1, ACTF.Tanh, scale=0.5,
                                     bias=bias_sb[f"b1h_{s}"][:, j:j + 1])
                a_sb = stage.tile([128, 512], F32, tag="asb")
                nc.vector.tensor_scalar_add(a_sb[:], ps1,
                                            bias_sb[f"b1_{s}"][:, j:j + 1])
                # act = (1+tanh)*(ps1+b1) = 2*silu(ps1+b1)
                nc.vector.scalar_tensor_tensor(
                    act[:], act[:], 1.0, a_sb[:], op0=ALU.add, op1=ALU.mult)
                dst = (qt_sb if s == "q" else kt_sb)[
                    :, j, t * 512:(t + 1) * 512]
                nc.vector.scalar_tensor_tensor(
                    dst, ps2, bias_sb[f"b2_{s}"][:, j:j + 1], act[:],
                    op0=ALU.add, op1=ALU.mult)

            def emit_v_item(t, j):
                """16+2 MMs + epilogue for seq block nt=t*4+j (v path)."""
                xt = x_tiles[("v", t)]
                w1sb, w2sb = w_sb["v"]
                pp = scps.tile([128, 2, 512], F32, tag="sc",
                               name=f"ppv{t}{j}")
                ps1, ps2 = pp[:, 0, :], pp[:, 1, :]
                jsl = slice(j * 128, (j + 1) * 128)
                for dc in range(8):
                    nc.tensor.matmul(ps1, xt[:, dc, jsl], w1sb[:, dc, :],
                                     start=(dc == 0), stop=False)
                    nc.tensor.matmul(ps2, xt[:, dc, jsl], w2sb[:, dc, :],
                                     start=(dc == 0), stop=False)
                nc.tensor.matmul(ps1, ones_r[:], b1vr[:],
                                 start=False, stop=True)
                nc.tensor.matmul(ps2, ones_r[:], b2vr[:],
                                 start=False, stop=True)
                act = stage.tile([128, 512], F32, tag="act")
                nc.scalar.activation(act[:], ps1, ACTF.Tanh, scale=0.5)
                # act = (1+tanh)*ps1 = 2*silu(ps1)
                nc.vector.scalar_tensor_tensor(
                    act[:], act[:], 1.0, ps1, op0=ALU.add, op1=ALU.mult)
                nt_i = t * 4 + j
                nc.vector.tensor_tensor(
                    v_sb[:, nt_i, :, 0:64],
                    ps2.rearrange("p (h d) -> p h d", h=8),
                    act[:].rearrange("p (h d) -> p h d", h=8),
                    ALU.mult)

            # ------------------- attention helpers ------------------------
            def emit_oproj_chunk(qg, ns, oh, ct_qg):
                nt_i = qg * 4 + ns
                nsl = slice(ns * 128, (ns + 1) * 128)
                pof = scps.tile([128, 2, 512], F32, tag="sc",
                                name=f"po{qg}{ns}{oh}")
                po = pof[:, 0, :]
                for j in range(4):
                    nc.tensor.matmul(
                        po, ct_qg[:, j, nsl],
                        woT_sb[:, j, oh * 512:(oh + 1) * 512],
                        start=(j == 0), stop=(j == 3))
                ot = ostage.tile([128, 512], F32, tag="ot")
                nc.vector.tensor_copy(ot[:], po)
                nc.sync.dma_start(
                    pout_d[nt_i * 128:(nt_i + 1) * 128,
                           oh * 512:(oh + 1) * 512],
                    ot[:])

            # ===================== main pipeline ==========================
            # q(0), k(0) projections up front (x DMAs pipelined by pools).
            for s in ("q", "k"):
                for jh in range(2):
                    for jj in range(2):
                        emit_qk_item(s, 0, jh, jj)
            if mask_mode != "causal":
                # attention for every q-group needs ALL of k and v: emit
                # the remaining projections up front, drip only q(t>0).
                for t in range(1, 4):
                    fetch_x("k", t)
                    for jh in range(2):
                        for jj in range(2):
                            emit_qk_item("k", t, jh, jj)
                for t in range(4):
                    fetch_x("v", t)
                    for j in range(4):
                        emit_v_item(t, j)

            pending = []          # deferred projection items (drip queue)
            oproj_pending = []    # deferred oproj chunks of previous qg
            norm_pending = []     # deferred normalization part-B closures

            def drip(n=1):
                for _ in range(n):
                    if pending:
                        pending.pop(0)()
                    elif oproj_pending:
                        oproj_pending.pop(0)()

            for qg in range(4):
                kcmax = kc_count(qg)
                qsl0 = qg * 512
                ct_qg = ctp.tile([128, 4, 512], BF16, tag="ct",
                                 name=f"ct{qg}")

                # stock the drip queue with next tile's q/k items; v(qg)
                # items are emitted in-line right before their first use.
                if qg < 3:
                    fetch_x("q", qg + 1)
                    if mask_mode == "causal":
                        fetch_x("k", qg + 1)
                        fetch_x("v", qg + 1)
                    srcs = ("q", "k") if mask_mode == "causal" else ("q",)
                    for s in srcs:
                        for jh in range(2):
                            for jj in range(2):
                                pending.append(
                                    (lambda s=s, t=qg + 1, jh=jh, jj=jj:
                                     emit_qk_item(s, t, jh, jj)))

                mtiles = None
                if mask_mode == "general":
                    mtiles = []
                    mt_sb = mpool.tile([128, NT, 512], BF16, tag="mt")
                    for kc in range(kcmax):
                        nc.sync.dma_start(
                            mt_sb[:, kc, :],
                            m01T_d[kc * 128:(kc + 1) * 128,
                                   qsl0:qsl0 + 512])
                        mtiles.append(mt_sb[:, kc, :])

                units = 0
                for pj in range(4):
                    ctx = cxps.tile([128, 2, 512], F32, tag="cx",
                                    name=f"ctx{qg}{pj}")
                    for kc in range(kcmax):
                        ksl = slice(kc * 128, (kc + 1) * 128)
                        diag_i = kc - 4 * qg
                        c0 = 128 * diag_i if (
                            mask_mode == "causal" and diag_i >= 0) else 0
                        ncols = 512 - c0
                        qca = slice(qsl0 + c0, qsl0 + 512)

                        diag = mask_mode == "causal" and diag_i >= 0
                        sc = scps.tile([128, 2, 512], F32, tag="sc",
                                       name=f"sc{qg}{pj}{kc}")
                        for par in range(2):
                            bp = par * 64
                            tp = (bp, 0) if SCORE_TILE_POS else None
                            nc.tensor.matmul(
                                sc[:, par, c0:],
                                kt_sb[bp:bp + 64, pj, ksl],
                                qt_sb[bp:bp + 64, pj, qca],
                                start=True, stop=not diag,
                                tile_position=tp)
                        if diag:
                            # add -1e9 to the masked (upper) triangle of the
                            # partial block so exp() yields exact zeros.
                            for par in range(2):
                                nc.tensor.matmul(
                                    sc[:, par, c0:c0 + 128],
                                    neye_sb[:], patc_sb[:],
                                    start=False, stop=True)

                        # v(qg) projection items drip in right after the
                        # first pj's scores that need them.
                        if pj == 0 and mask_mode == "causal" and diag_i >= 0:
                            emit_v_item(qg, diag_i)

                        attn = attnp.tile([128, 2, 512], BF16, tag="at",
                                          name=f"at{qg}{pj}{kc}")
                        nc.scalar.activation(attn[:, :, c0:], sc[:, :, c0:],
                                             ACTF.Exp)
                        if mask_mode == "general":
                            nc.gpsimd.tensor_tensor(
                                attn[:], attn[:],
                                mtiles[kc][:, None, :].to_broadcast(
                                    [128, 2, 512]),
                                ALU.mult)
                        for par in range(2):
                            hl = 2 * pj + par
                            nc.tensor.matmul(
                                ctx[0:65, par, c0:],
                                v_sb[:, kc, hl, :],
                                attn[:, par, c0:],
                                start=(kc == 0),
                                stop=(kc == kcmax - 1))
                        units += 1
                        if kc == 2 and norm_pending:
                            norm_pending.pop(0)()
                        if units % DRIP_EVERY == 0:
                            drip()

                    # fill the PE queue before the serial norm chain:
                    # oproj chunks of the previous q-group + one drip item.
                    for _ in range(2):
                        if oproj_pending:
                            oproj_pending.pop(0)()
                    if pending:
                        pending.pop(0)()

                    # ---- normalization, part A: DVE reciprocal chain ----
                    nc.vector.tensor_copy(den_sb[0:1, :], ctx[64:65, 0, :])
                    nc.vector.tensor_copy(den_sb[32:33, :], ctx[64:65, 1, :])
                    nc.vector.reciprocal_approx_fast(rec_sb[:], den_sb[:])
                    nc.vector.tensor_copy(rec_bf[:], rec_sb[:])

                    # part B (bc matmul + ct multiplies) is deferred into the
                    # next head pair's kc loop so the PE FIFO never waits on
                    # the DVE chain above.
                    def emit_normB(qg=qg, pj=pj, ctx=ctx, ct_qg=ct_qg):
                        bc_full = scps.tile([128, 2, 512], F32, tag="sc",
                                            name=f"bc{qg}{pj}")
                        bc_ps = bc_full[:, 0, :]
                        nc.tensor.matmul(bc_ps, sel2[:], rec_bf[:])
                        bc_sb = smalls.tile([128, 512], BF16, tag="bcs")
                        nc.vector.tensor_copy(bc_sb[:], bc_ps)
                        for par in range(2):
                            bp = par * 64
                            nc.vector.tensor_tensor(
                                ct_qg[bp:bp + 64, pj, :],
                                ctx[0:64, par, :],
                                bc_sb[bp:bp + 64, :], ALU.mult)
                    norm_pending.append(emit_normB)

                # flush remaining projection items before the next q-group
                while pending:
                    drip()
                while norm_pending:
                    norm_pending.pop(0)()
                while oproj_pending:
                    oproj_pending.pop(0)()
                for ns in range(4):
                    for oh in range(2):
                        oproj_pending.append(
                            (lambda qg=qg, ns=ns, oh=oh, ct=ct_qg:
                             emit_oproj_chunk(qg, ns, oh, ct)))

            while oproj_pending:
                oproj_pending.pop(0)()
    nc.compile()
    return nc


def _host_prepare(inputs):
    """Split the full problem into 8 per-core input maps + host-side info."""
    q = np.asarray(inputs["query"], dtype=np.float32)
    k = np.asarray(inputs["key"], dtype=np.float32)
    v = np.asarray(inputs["value"], dtype=np.float32)
    mask = np.asarray(inputs["mask"])
    w = {n: np.asarray(inputs[n], dtype=np.float32)
         for n in ("wq1", "wq2", "wk1", "wk2", "wv1", "wv2", "wo")}
    bias = {n: np.asarray(inputs[n], dtype=np.float32)
            for n in ("bq1", "bq2", "bk1", "bk2", "bv1", "bv2", "bo")}

    m = mask.reshape(S, S)
    if np.array_equal(m != 0, np.tril(np.ones((S, S), bool))):
        mask_mode = "causal"
    elif np.all(m != 0):
        mask_mode = "full"
    else:
        mask_mode = "general"

    patc = None
    m01T = None
    if mask_mode == "causal":
        kk = np.arange(128)[:, None]
        qq = np.arange(128)[None, :]
        patc = np.ascontiguousarray((kk > qq).astype(np.float32))
    elif mask_mode == "general":
        m01T = np.ascontiguousarray((m != 0).T.astype(np.float32))

    scale = 1.0 / np.sqrt(DK).astype(np.float32)

    import ml_dtypes

    def cvt(a):
        return np.ascontiguousarray(a).astype(ml_dtypes.bfloat16)

    in_maps = []
    for c in range(NCORES):
        b, g = divmod(c, 2)
        sl = slice(g * GCH, (g + 1) * GCH)
        def xprep(a):
            # [D, S] -> [128 p, 4 t, 8 dc, 512 o]; x[dc*128+p, t*512+o]
            return cvt(a.T.reshape(8, 128, 4, 512).transpose(1, 2, 0, 3))

        def wprep(a):
            # [D, GCH] -> [128 p, 8 dc, 512 o]; w[dc*128+p, o]
            return cvt(a.reshape(8, 128, GCH).transpose(1, 0, 2))

        im = {
            "xqT": xprep(q[b]),
            "xkT": xprep(k[b]),
            "xvT": xprep(v[b]),
            "w1T_q": wprep(w["wq1"][sl].T),
            # fold the 1/sqrt(dk) score scale into the non-silu Q branch,
            # and 0.5 everywhere (silu computed as A*(1+tanh(A/2)) = 2*silu)
            "w2T_q": wprep(w["wq2"][sl].T * (scale * 0.5)),
            "w2T_k": wprep(w["wk2"][sl].T * 0.5),
            "w2T_v": wprep(w["wv2"][sl].T * 0.5),
            "w1T_k": wprep(w["wk1"][sl].T),
            "w1T_v": wprep(w["wv1"][sl].T),
            "b1_q": np.ascontiguousarray(bias["bq1"][sl].reshape(4, 128).T),
            "b1h_q": np.ascontiguousarray(
                (bias["bq1"][sl] * 0.5).reshape(4, 128).T),
            "b2_q": np.ascontiguousarray(
                (bias["bq2"][sl] * (scale * 0.5)).reshape(4, 128).T),
            "b1_k": np.ascontiguousarray(bias["bk1"][sl].reshape(4, 128).T),
            "b1h_k": np.ascontiguousarray(
                (bias["bk1"][sl] * 0.5).reshape(4, 128).T),
            "b2_k": np.ascontiguousarray(
                (bias["bk2"][sl] * 0.5).reshape(4, 128).T),
            "b1_v": cvt(bias["bv1"][sl].reshape(1, GCH)),
            "b2_v": cvt((bias["bv2"][sl] * 0.5).reshape(1, GCH)),
            "woT": cvt(
                w["wo"][:, sl].T.reshape(4, 128, D).transpose(1, 0, 2)),
        }
        if mask_mode == "causal":
            im["patc"] = cvt(patc)
            im["neye"] = cvt(-1e9 * np.eye(128, dtype=np.float32))
        elif mask_mode == "general":
            im["m01T"] = cvt(m01T)
        in_maps.append(im)
    return mask_mode, in_maps, bias["bo"]


def kernel(**inputs):
    global LAST_RESULT
    mask_mode, in_maps, bo = _host_prepare(inputs)
    nc = build_program(mask_mode)

    import concourse.bass_utils as bu

    if TRACE:
        import types

        try:
            from trn_agent_boot.trn_boot import _ntff_profile_via_ctypes

            hook = _ntff_profile_via_ctypes("/opt/axon/libaxon_pjrt.so")
            m = types.ModuleType("antenv.axon_hooks")
            m.get_axon_ntff_profile_hook = lambda: hook
            import antenv  # noqa: F401

            sys.modules["antenv.axon_hooks"] = m
            bu.upload_artifacts = lambda d: "local://skipped"
        except Exception as e:
            print("profiling hook install failed:", e)

    res = bu.run_bass_kernel_spmd(
        nc, in_maps, core_ids=list(range(NCORES)),
        trace=TRACE, trace_cores=TRACE_CORES,
    )
    LAST_RESULT = res

    out = np.empty((B, S, D), dtype=np.float32)
    for b in range(B):
        out[b] = (res.results[2 * b]["pout"] + res.results[2 * b + 1]["pout"]
                  + bo[None, :])
    return out


# revision 20
# speedup vs baseline: 1.1372x; 1.0023x over previous
"""SwiGLU-projected causal MHA (B=4, S=2048, D=1024, H=16) on 8 TRN2 NeuronCores.

Sharding: core c -> (batch b = c//2, head-group g = c%2).  Each core computes
the SwiGLU Q/K/V projections for its 512 output channels (= 8 heads) of its
batch, runs causal attention for those heads, and produces a partial output
projection (contraction over its 512 channels).  The host sums the two
partials per batch and adds the output bias.

v2: software-pipelined emission.  Projections are split into fine-grained
items; attention for q-group t starts right after q(t)/k(t) are projected,
and the remaining projection items (v(t), q(t+1), k(t+1)) are dripped into
the attention kc loops to fill the PE idle created by the ACT-bound exp
stream.  Score matmul pairs use explicit row-group tile_position for PE
concurrency; exp runs once per kc over both heads of a pair via a strided
2-bank PSUM read; diagonal blocks are column-truncated (causal); the partial
mask multiply runs on the Pool engine; softmax normalization uses
reciprocal_approx_fast plus a single selector matmul to broadcast both
reciprocal rows in one shot.
"""
import sys

sys.path.insert(0, "/opt/trn_rl_repo")
import numpy as np

import concourse.bacc as bacc
import concourse.tile as tile
import concourse.mybir as mybir

B, S, D = 4, 2048, 1024
H, DK = 16, 64
NCORES = 8
GCH = 512          # channels per core (8 heads)
NT = S // 128      # 16 seq chunks
F32 = mybir.dt.float32
F32R = mybir.dt.float32r
BF16 = mybir.dt.bfloat16
ACTF = mybir.ActivationFunctionType
ALU = mybir.AluOpType

TRACE = False          # set by test.py for profiling runs
TRACE_CORES = None
LAST_RESULT = None     # BassKernelResults stash for test.py

DRIP_EVERY = 5         # attention kc-units between dripped projection items
SCORE_TILE_POS = True  # explicit row-group tile_position on score MM pairs


def build_program(mask_mode):
    """mask_mode: 'causal' (tril), 'full' (all ones), 'general' (arbitrary)."""
    nc = bacc.Bacc("TRN2", target_bir_lowering=False, debug=False)

    xT = {s: nc.dram_tensor(f"x{s}T", [128, 4, 8, 512], BF16,
                            kind="ExternalInput")
          for s in "qkv"}
    w1T = {s: nc.dram_tensor(f"w1T_{s}", [128, 8, GCH], BF16,
                             kind="ExternalInput")
           for s in "qkv"}
    w2T = {s: nc.dram_tensor(f"w2T_{s}", [128, 8, GCH], BF16,
                             kind="ExternalInput")
           for s in "qkv"}
    bias_d = {}
    for s in "qk":
        for bn in ("b1", "b2", "b1h"):
            bias_d[f"{bn}_{s}"] = nc.dram_tensor(f"{bn}_{s}", [128, 4], F32,
                                                 kind="ExternalInput")
    b1v_d = nc.dram_tensor("b1_v", [1, GCH], BF16, kind="ExternalInput")
    b2v_d = nc.dram_tensor("b2_v", [1, GCH], BF16, kind="ExternalInput")
    woT_d = nc.dram_tensor("woT", [128, 4, D], BF16, kind="ExternalInput")
    patc_d = neye_d = m01T_d = None
    if mask_mode == "causal":
        patc_d = nc.dram_tensor("patc", [128, 128], BF16, kind="ExternalInput")
        neye_d = nc.dram_tensor("neye", [128, 128], BF16, kind="ExternalInput")
    elif mask_mode == "general":
        m01T_d = nc.dram_tensor("m01T", [S, S], BF16, kind="ExternalInput")
    pout_d = nc.dram_tensor("pout", [S, D], F32, kind="ExternalOutput")

    def kc_count(qg):
        return 4 * qg + 4 if mask_mode == "causal" else NT

    with tile.TileContext(nc) as tc:
        with (
            tc.tile_pool(name="persist", bufs=1) as persist,
            tc.tile_pool(name="xq", bufs=2) as xq_pool,
            tc.tile_pool(name="xk", bufs=2) as xk_pool,
            tc.tile_pool(name="xv", bufs=2) as xv_pool,
            tc.tile_pool(name="stage", bufs=3) as stage,
            tc.tile_pool(name="attnp", bufs=3) as attnp,
            tc.tile_pool(name="ctp", bufs=2) as ctp,
            tc.tile_pool(name="smalls", bufs=2) as smalls,
            tc.tile_pool(name="ostage", bufs=2) as ostage,
            tc.tile_pool(name="mpool", bufs=2) as mpool,
            tc.tile_pool(name="scps", bufs=2, space="PSUM") as scps,
            tc.tile_pool(name="cxps", bufs=2, space="PSUM") as cxps,
        ):
            xpools = {"q": xq_pool, "k": xk_pool, "v": xv_pool}

            # ---------------- persistent tiles + upfront DMAs ------------
            # DMA emission order tracks first use: x(q,0) + w_q first.
            qt_sb = persist.tile([128, 4, S], BF16, tag="qt")
            kt_sb = persist.tile([128, 4, S], BF16, tag="kt")
            v_sb = persist.tile([128, NT, 8, 65], BF16, tag="v")

            x_tiles = {}

            def fetch_x(s, t):
                if (s, t) in x_tiles:
                    return x_tiles[(s, t)]
                xt = xpools[s].tile([128, 8, 512], BF16, tag=f"x{s}")
                for h in range(4):
                    nc.sync.dma_start(
                        xt[:, 2 * h:2 * h + 2, :],
                        xT[s][:, t, 2 * h:2 * h + 2, :],
                    )
                x_tiles[(s, t)] = xt
                return xt

            w_sb = {}

            def fetch_w(s):
                w1sb = persist.tile([128, 8, GCH], BF16, tag=f"w1{s}")
                w2sb = persist.tile([128, 8, GCH], BF16, tag=f"w2{s}")
                for wsb, wd in ((w1sb, w1T[s]), (w2sb, w2T[s])):
                    for h in range(4):
                        nc.sync.dma_start(
                            wsb[:, 2 * h:2 * h + 2, :],
                            wd[:, 2 * h:2 * h + 2, :])
                w_sb[s] = (w1sb, w2sb)

            fetch_x("q", 0)
            fetch_w("q")
            fetch_x("k", 0)
            fetch_w("k")
            fetch_x("v", 0)
            fetch_w("v")

            bias_sb = {}
            for s in "qk":
                for bn in ("b1", "b2", "b1h"):
                    t_ = persist.tile([128, 4], F32, tag=f"{bn}{s}")
                    nc.sync.dma_start(t_[:], bias_d[f"{bn}_{s}"][:])
                    bias_sb[f"{bn}_{s}"] = t_
            b1vr = persist.tile([1, GCH], BF16, tag="b1v")
            b2vr = persist.tile([1, GCH], BF16, tag="b2v")
            nc.sync.dma_start(b1vr[:], b1v_d[:])
            nc.sync.dma_start(b2vr[:], b2v_d[:])
            woT_sb = persist.tile([128, 4, D], BF16, tag="wo")
            nc.sync.dma_start(woT_sb[:], woT_d[:])

            onesf = persist.tile([1, 128], F32, tag="onesf")
            ones_r = persist.tile([1, 128], BF16, tag="ones_r")
            nc.any.memset(onesf[:], 1.0)
            nc.vector.tensor_copy(ones_r[:], onesf[:])
            onescol = persist.tile([128, 1], F32, tag="onescol")
            nc.any.memset(onescol[:], 1.0)
            nc.vector.tensor_copy(
                v_sb[:, :, :, 64:65],
                onescol[:, None, :].to_broadcast([128, NT, 8, 1]),
            )
            # selector for the denominator broadcast: one K=33 matmul puts
            # rec row 0 on out rows 0..63 and rec row 32 on rows 64..127.
            # (partition bases must be 32-aligned, hence rows 0/32; the
            # pad rows stay memset so 0*pad contributes exact zeros.)
            sel2 = persist.tile([33, 128], BF16, tag="sel2")
            nc.any.memset(sel2[:], 0.0)
            nc.any.memset(sel2[0:1, 0:64], 1.0)
            nc.any.memset(sel2[32:33, 64:128], 1.0)
            den_sb = persist.tile([33, 512], F32, tag="den")
            nc.any.memset(den_sb[:], 1.0)
            rec_sb = persist.tile([33, 512], F32, tag="rec")
            nc.any.memset(rec_sb[:], 1.0)
            rec_bf = persist.tile([33, 512], BF16, tag="recbf")
            nc.any.memset(rec_bf[:], 1.0)

            patc_sb = neye_sb = None
            if mask_mode == "causal":
                patc_sb = persist.tile([128, 128], BF16, tag="patc")
                nc.sync.dma_start(patc_sb[:], patc_d[:])
                neye_sb = persist.tile([128, 128], BF16, tag="neye")
                nc.sync.dma_start(neye_sb[:], neye_d[:])

            # ------------------- projection items ------------------------
            def emit_qk_item(s, t, jh, jj):
                """16 MMs + epilogue for channel chunk j=jh*2+jj, seq tile t."""
                j = jh * 2 + jj
                xt = x_tiles[(s, t)]
                w1sb, w2sb = w_sb[s]
                pp = scps.tile([128, 2, 512], F32, tag="sc",
                               name=f"pp{s}{t}{j}")
                ps1, ps2 = pp[:, 0, :], pp[:, 1, :]
                jsl = slice(j * 128, (j + 1) * 128)
                for dc in range(8):
                    nc.tensor.matmul(ps1, w1sb[:, dc, jsl], xt[:, dc, :],
                                     start=(dc == 0), stop=(dc == 7))
                    nc.tensor.matmul(ps2, w2sb[:, dc, jsl], xt[:, dc, :],
                                     start=(dc == 0), stop=(dc == 7))
                act = stage.tile([128, 512], F32, tag="act")
                # act = tanh((ps1+b1)/2)
                nc.scalar.activation(act[:], ps1, ACTF.Tanh, scale=0.5,
                                     bias=bias_sb[f"b1h_{s}"][:, j:j + 1])
                a_sb = stage.tile([128, 512], F32, tag="asb")
                nc.vector.tensor_scalar_add(a_sb[:], ps1,
                                            bias_sb[f"b1_{s}"][:, j:j + 1])
                # act = (1+tanh)*(ps1+b1) = 2*silu(ps1+b1)
                nc.vector.scalar_tensor_tensor(
                    act[:], act[:], 1.0, a_sb[:], op0=ALU.add, op1=ALU.mult)
                dst = (qt_sb if s == "q" else kt_sb)[
                    :, j, t * 512:(t + 1) * 512]
                nc.vector.scalar_tensor_tensor(
                    dst, ps2, bias_sb[f"b2_{s}"][:, j:j + 1], act[:],
                    op0=ALU.add, op1=ALU.mult)

            def emit_v_item(t, j):
                """16+2 MMs + epilogue for seq block nt=t*4+j (v path)."""
                xt = x_tiles[("v", t)]
                w1sb, w2sb = w_sb["v"]
                pp = scps.tile([128, 2, 512], F32, tag="sc",
                               name=f"ppv{t}{j}")
                ps1, ps2 = pp[:, 0, :], pp[:, 1, :]
                jsl = slice(j * 128, (j + 1) * 128)
                for dc in range(8):
                    nc.tensor.matmul(ps1, xt[:, dc, jsl], w1sb[:, dc, :],
                                     start=(dc == 0), stop=False)
                    nc.tensor.matmul(ps2, xt[:, dc, jsl], w2sb[:, dc, :],
                                     start=(dc == 0), stop=False)
                nc.tensor.matmul(ps1, ones_r[:], b1vr[:],
                                 start=False, stop=True)
                nc.tensor.matmul(ps2, ones_r[:], b2vr[:],
                                 start=False, stop=True)
                act = stage.tile([128, 512], F32, tag="act")
                nc.scalar.activation(act[:], ps1, ACTF.Tanh, scale=0.5)
                # act = (1+tanh)*ps1 = 2*silu(ps1)
                nc.vector.scalar_tensor_tensor(
                    act[:], act[:], 1.0, ps1, op0=ALU.add, op1=ALU.mult)
                nt_i = t * 4 + j
                nc.vector.tensor_tensor(
                    v_sb[:, nt_i, :, 0:64],
                    ps2.rearrange("p (h d) -> p h d", h=8),
                    act[:].rearrange("p (h d) -> p h d", h=8),
                    ALU.mult)

            # ------------------- attention helpers ------------------------
            def emit_oproj_chunk(qg, ns, oh, ct_qg):
                nt_i = qg * 4 + ns
                nsl = slice(ns * 128, (ns + 1) * 128)
                pof = scps.tile([128, 2, 512], F32, tag="sc",
                                name=f"po{qg}{ns}{oh}")
                po = pof[:, 0, :]
                for j in range(4):
                    nc.tensor.matmul(
                        po, ct_qg[:, j, nsl],
                        woT_sb[:, j, oh * 512:(oh + 1) * 512],
                        start=(j == 0), stop=(j == 3))
                ot = ostage.tile([128, 512], F32, tag="ot")
                nc.vector.tensor_copy(ot[:], po)
                nc.sync.dma_start(
                    pout_d[nt_i * 128:(nt_i + 1) * 128,
                           oh * 512:(oh + 1) * 512],
                    ot[:])

            # ===================== main pipeline ==========================
            # q(0), k(0) projections up front (x DMAs pipelined by pools).
            for s in ("q", "k"):
                for jh in range(2):
                    for jj in range(2):
                        emit_qk_item(s, 0, jh, jj)
            if mask_mode != "causal":
                # attention for every q-group needs ALL of k and v: emit
                # the remaining projections up front, drip only q(t>0).
                for t in range(1, 4):
                    fetch_x("k", t)
                    for jh in range(2):
                        for jj in range(2):
                            emit_qk_item("k", t, jh, jj)
                for t in range(4):
                    fetch_x("v", t)
                    for j in range(4):
                        emit_v_item(t, j)

            pending = []          # deferred projection items (drip queue)
            oproj_pending = []    # deferred oproj chunks of previous qg
            norm_pending = []     # deferred normalization part-B closures

            def drip(n=1):
                for _ in range(n):
                    if pending:
                        pending.pop(0)()
                    elif oproj_pending:
                        oproj_pending.pop(0)()

            for qg in range(4):
                kcmax = kc_count(qg)
                qsl0 = qg * 512
                ct_qg = ctp.tile([128, 4, 512], BF16, tag="ct",
                                 name=f"ct{qg}")

                # stock the drip queue with next tile's q/k items; v(qg)
                # items are emitted in-line right before their first use.
                if qg < 3:
                    fetch_x("q", qg + 1)
                    if mask_mode == "causal":
                        fetch_x("k", qg + 1)
                        fetch_x("v", qg + 1)
                    srcs = ("q", "k") if mask_mode == "causal" else ("q",)
                    for s in srcs:
                        for jh in range(2):
                            for jj in range(2):
                                pending.append(
                                    (lambda s=s, t=qg + 1, jh=jh, jj=jj:
                                     emit_qk_item(s, t, jh, jj)))

                mtiles = None
                if mask_mode == "general":
                    mtiles = []
                    mt_sb = mpool.tile([128, NT, 512], BF16, tag="mt")
                    for kc in range(kcmax):
                        nc.sync.dma_start(
                            mt_sb[:, kc, :],
                            m01T_d[kc * 128:(kc + 1) * 128,
                                   qsl0:qsl0 + 512])
                        mtiles.append(mt_sb[:, kc, :])

                units = 0
                for pj in range(4):
                    ctx = cxps.tile([128, 2, 512], F32, tag="cx",
                                    name=f"ctx{qg}{pj}")
                    for kc in range(kcmax):
                        ksl = slice(kc * 128, (kc + 1) * 128)
                        diag_i = kc - 4 * qg
                        c0 = 128 * diag_i if (
                            mask_mode == "causal" and diag_i >= 0) else 0
                        ncols = 512 - c0
                        qca = slice(qsl0 + c0, qsl0 + 512)

                        diag = mask_mode == "causal" and diag_i >= 0
                        sc = scps.tile([128, 2, 512], F32, tag="sc",
                                       name=f"sc{qg}{pj}{kc}")
                        for par in range(2):
                            bp = par * 64
                            tp = (bp, 0) if SCORE_TILE_POS else None
                            nc.tensor.matmul(
                                sc[:, par, c0:],
                                kt_sb[bp:bp + 64, pj, ksl],
                                qt_sb[bp:bp + 64, pj, qca],
                                start=True, stop=not diag,
                                tile_position=tp)
                        if diag:
                            # add -1e9 to the masked (upper) triangle of the
                            # partial block so exp() yields exact zeros.
                            for par in range(2):
                                nc.tensor.matmul(
                                    sc[:, par, c0:c0 + 128],
                                    neye_sb[:], patc_sb[:],
                                    start=False, stop=True)

                        # v(qg) projection items drip in right after the
                        # first pj's scores that need them.
                        if pj == 0 and mask_mode == "causal" and diag_i >= 0:
                            emit_v_item(qg, diag_i)

                        attn = attnp.tile([128, 2, 512], BF16, tag="at",
                                          name=f"at{qg}{pj}{kc}")
                        nc.scalar.activation(attn[:, :, c0:], sc[:, :, c0:],
                                             ACTF.Exp)
                        if mask_mode == "general":
                            nc.gpsimd.tensor_tensor(
                                attn[:], attn[:],
                                mtiles[kc][:, None, :].to_broadcast(
                                    [128, 2, 512]),
                                ALU.mult)
                        for par in range(2):
                            hl = 2 * pj + par
                            nc.tensor.matmul(
                                ctx[0:65, par, c0:],
                                v_sb[:, kc, hl, :],
                                attn[:, par, c0:],
                                start=(kc == 0),
                                stop=(kc == kcmax - 1))
                        units += 1
                        if kc == 2 and norm_pending:
                            norm_pending.pop(0)()
                        if units % DRIP_EVERY == 0:
                            drip()

                    # fill the PE queue before the serial norm chain:
                    # oproj chunks of the previous q-group + one drip item.
                    for _ in range(2):
                        if oproj_pending:
                            oproj_pending.pop(0)()
                    if pending:
                        pending.pop(0)()

                    # ---- normalization, part A: DVE reciprocal chain ----
                    nc.vector.tensor_copy(den_sb[0:1, :], ctx[64:65, 0, :])
                    nc.vector.tensor_copy(den_sb[32:33, :], ctx[64:65, 1, :])
                    nc.vector.reciprocal_approx_fast(rec_sb[:], den_sb[:])
                    nc.vector.tensor_copy(rec_bf[:], rec_sb[:])

                    # part B (bc matmul + ct multiplies) is deferred into the
                    # next head pair's kc loop so the PE FIFO never waits on
                    # the DVE chain above.
                    def emit_normB(qg=qg, pj=pj, ctx=ctx, ct_qg=ct_qg):
                        bc_full = scps.tile([128, 2, 512], F32, tag="sc",
                                            name=f"bc{qg}{pj}")
                        bc_ps = bc_full[:, 0, :]
                        nc.tensor.matmul(bc_ps, sel2[:], rec_bf[:])
                        bc_sb = smalls.tile([128, 512], BF16, tag="bcs")
                        nc.vector.tensor_copy(bc_sb[:], bc_ps)
                        for par in range(2):
                            bp = par * 64
                            nc.vector.tensor_tensor(
                                ct_qg[bp:bp + 64, pj, :],
                                ctx[0:64, par, :],
                                bc_sb[bp:bp + 64, :], ALU.mult)
                    norm_pending.append(emit_normB)

                # flush remaining projection items before the next q-group.
                # normB of the last head pair stays deferred into the next
                # q-group's first kc loop (it would block the PE FIFO here).
                while pending:
                    drip()
                if qg == 3:
                    while norm_pending:
                        norm_pending.pop(0)()
                while oproj_pending:
                    oproj_pending.pop(0)()
                for ns in range(4):
                    for oh in range(2):
                        oproj_pending.append(
                            (lambda qg=qg, ns=ns, oh=oh, ct=ct_qg:
                             emit_oproj_chunk(qg, ns, oh, ct)))

            while oproj_pending:
                oproj_pending.pop(0)()
    nc.compile()
    return nc


def _host_prepare(inputs):
    """Split the full problem into 8 per-core input maps + host-side info."""
    q = np.asarray(inputs["query"], dtype=np.float32)
    k = np.asarray(inputs["key"], dtype=np.float32)
    v = np.asarray(inputs["value"], dtype=np.float32)
    mask = np.asarray(inputs["mask"])
    w = {n: np.asarray(inputs[n], dtype=np.float32)
         for n in ("wq1", "wq2", "wk1", "wk2", "wv1", "wv2", "wo")}
    bias = {n: np.asarray(inputs[n], dtype=np.float32)
            for n in ("bq1", "bq2", "bk1", "bk2", "bv1", "bv2", "bo")}

    m = mask.reshape(S, S)
    if np.array_equal(m != 0, np.tril(np.ones((S, S), bool))):
        mask_mode = "causal"
    elif np.all(m != 0):
        mask_mode = "full"
    else:
        mask_mode = "general"

    patc = None
    m01T = None
    if mask_mode == "causal":
        kk = np.arange(128)[:, None]
        qq = np.arange(128)[None, :]
        patc = np.ascontiguousarray((kk > qq).astype(np.float32))
    elif mask_mode == "general":
        m01T = np.ascontiguousarray((m != 0).T.astype(np.float32))

    scale = 1.0 / np.sqrt(DK).astype(np.float32)

    import ml_dtypes

    def cvt(a):
        return np.ascontiguousarray(a).astype(ml_dtypes.bfloat16)

    in_maps = []
    for c in range(NCORES):
        b, g = divmod(c, 2)
        sl = slice(g * GCH, (g + 1) * GCH)
        def xprep(a):
            # [D, S] -> [128 p, 4 t, 8 dc, 512 o]; x[dc*128+p, t*512+o]
            return cvt(a.T.reshape(8, 128, 4, 512).transpose(1, 2, 0, 3))

        def wprep(a):
            # [D, GCH] -> [128 p, 8 dc, 512 o]; w[dc*128+p, o]
            return cvt(a.reshape(8, 128, GCH).transpose(1, 0, 2))

        im = {
            "xqT": xprep(q[b]),
            "xkT": xprep(k[b]),
            "xvT": xprep(v[b]),
            "w1T_q": wprep(w["wq1"][sl].T),
            # fold the 1/sqrt(dk) score scale into the non-silu Q branch,
            # and 0.5 everywhere (silu computed as A*(1+tanh(A/2)) = 2*silu)
            "w2T_q": wprep(w["wq2"][sl].T * (scale * 0.5)),
            "w2T_k": wprep(w["wk2"][sl].T * 0.5),
            "w2T_v": wprep(w["wv2"][sl].T * 0.5),
            "w1T_k": wprep(w["wk1"][sl].T),
            "w1T_v": wprep(w["wv1"][sl].T),
            "b1_q": np.ascontiguousarray(bias["bq1"][sl].reshape(4, 128).T),
            "b1h_q": np.ascontiguousarray(
                (bias["bq1"][sl] * 0.5).reshape(4, 128).T),
            "b2_q": np.ascontiguousarray(
                (bias["bq2"][sl] * (scale * 0.5)).reshape(4, 128).T),
            "b1_k": np.ascontiguousarray(bias["bk1"][sl].reshape(4, 128).T),
            "b1h_k": np.ascontiguousarray(
                (bias["bk1"][sl] * 0.5).reshape(4, 128).T),
            "b2_k": np.ascontiguousarray(
                (bias["bk2"][sl] * 0.5).reshape(4, 128).T),
            "b1_v": cvt(bias["bv1"][sl].reshape(1, GCH)),
            "b2_v": cvt((bias["bv2"][sl] * 0.5).reshape(1, GCH)),
            "woT": cvt(
                w["wo"][:, sl].T.reshape(4, 128, D).transpose(1, 0, 2)),
        }
        if mask_mode == "causal":
            im["patc"] = cvt(patc)
            im["neye"] = cvt(-1e9 * np.eye(128, dtype=np.float32))
        elif mask_mode == "general":
            im["m01T"] = cvt(m01T)
        in_maps.append(im)
    return mask_mode, in_maps, bias["bo"]


def kernel(**inputs):
    global LAST_RESULT
    mask_mode, in_maps, bo = _host_prepare(inputs)
    nc = build_program(mask_mode)

    import concourse.bass_utils as bu

    if TRACE:
        import types

        try:
            from trn_agent_boot.trn_boot import _ntff_profile_via_ctypes

            hook = _ntff_profile_via_ctypes("/opt/axon/libaxon_pjrt.so")
            m = types.ModuleType("antenv.axon_hooks")
            m.get_axon_ntff_profile_hook = lambda: hook
            import antenv  # noqa: F401

            sys.modules["antenv.axon_hooks"] = m
            bu.upload_artifacts = lambda d: "local://skipped"
        except Exception as e:
            print("profiling hook install failed:", e)

    res = bu.run_bass_kernel_spmd(
        nc, in_maps, core_ids=list(range(NCORES)),
        trace=TRACE, trace_cores=TRACE_CORES,
    )
    LAST_RESULT = res

    out = np.empty((B, S, D), dtype=np.float32)
    for b in range(B):
        out[b] = (res.results[2 * b]["pout"] + res.results[2 * b + 1]["pout"]
                  + bo[None, :])
    return out


# revision 22
# speedup vs baseline: 1.1678x; 1.0269x over previous
"""SwiGLU-projected causal MHA (B=4, S=2048, D=1024, H=16) on 8 TRN2 NeuronCores.

Sharding: core c -> (batch b = c//2, head-group g = c%2).  Each core computes
the SwiGLU Q/K/V projections for its 512 output channels (= 8 heads) of its
batch, runs causal attention for those heads, and produces a partial output
projection (contraction over its 512 channels).  The host sums the two
partials per batch and adds the output bias.

v2: software-pipelined emission.  Projections are split into fine-grained
items; attention for q-group t starts right after q(t)/k(t) are projected,
and the remaining projection items (v(t), q(t+1), k(t+1)) are dripped into
the attention kc loops to fill the PE idle created by the ACT-bound exp
stream.  Score matmul pairs use explicit row-group tile_position for PE
concurrency; exp runs once per kc over both heads of a pair via a strided
2-bank PSUM read; diagonal blocks are column-truncated (causal); the partial
mask multiply runs on the Pool engine; softmax normalization uses
reciprocal_approx_fast plus a single selector matmul to broadcast both
reciprocal rows in one shot.
"""
import sys

sys.path.insert(0, "/opt/trn_rl_repo")
import numpy as np

import concourse.bacc as bacc
import concourse.tile as tile
import concourse.mybir as mybir

B, S, D = 4, 2048, 1024
H, DK = 16, 64
NCORES = 8
GCH = 512          # channels per core (8 heads)
NT = S // 128      # 16 seq chunks
F32 = mybir.dt.float32
F32R = mybir.dt.float32r
BF16 = mybir.dt.bfloat16
ACTF = mybir.ActivationFunctionType
ALU = mybir.AluOpType

TRACE = False          # set by test.py for profiling runs
TRACE_CORES = None
LAST_RESULT = None     # BassKernelResults stash for test.py

DRIP_EVERY = 5         # attention kc-units between dripped projection items
SCORE_TILE_POS = True  # explicit row-group tile_position on score MM pairs


def build_program(mask_mode):
    """mask_mode: 'causal' (tril), 'full' (all ones), 'general' (arbitrary)."""
    nc = bacc.Bacc("TRN2", target_bir_lowering=False, debug=False)

    xT = {s: nc.dram_tensor(f"x{s}T", [128, 4, 8, 512], BF16,
                            kind="ExternalInput")
          for s in "qkv"}
    w1T = {s: nc.dram_tensor(f"w1T_{s}", [128, 8, GCH], BF16,
                             kind="ExternalInput")
           for s in "qkv"}
    w2T = {s: nc.dram_tensor(f"w2T_{s}", [128, 8, GCH], BF16,
                             kind="ExternalInput")
           for s in "qkv"}
    bias_d = {}
    for s in "qk":
        for bn in ("b1", "b2", "b1h"):
            bias_d[f"{bn}_{s}"] = nc.dram_tensor(f"{bn}_{s}", [128, 4], F32,
                                                 kind="ExternalInput")
    b1v_d = nc.dram_tensor("b1_v", [1, GCH], BF16, kind="ExternalInput")
    b2v_d = nc.dram_tensor("b2_v", [1, GCH], BF16, kind="ExternalInput")
    woT_d = nc.dram_tensor("woT", [128, 4, D], BF16, kind="ExternalInput")
    patc_d = neye_d = m01T_d = None
    if mask_mode == "causal":
        patc_d = nc.dram_tensor("patc", [128, 128], BF16, kind="ExternalInput")
        neye_d = nc.dram_tensor("neye", [128, 128], BF16, kind="ExternalInput")
    elif mask_mode == "general":
        m01T_d = nc.dram_tensor("m01T", [S, S], BF16, kind="ExternalInput")
    pout_d = nc.dram_tensor("pout", [S, D], F32, kind="ExternalOutput")

    def kc_count(qg):
        return 4 * qg + 4 if mask_mode == "causal" else NT

    with tile.TileContext(nc) as tc:
        with (
            tc.tile_pool(name="persist", bufs=1) as persist,
            tc.tile_pool(name="xq", bufs=2) as xq_pool,
            tc.tile_pool(name="xk", bufs=2) as xk_pool,
            tc.tile_pool(name="xv", bufs=2) as xv_pool,
            tc.tile_pool(name="stage", bufs=3) as stage,
            tc.tile_pool(name="attnp", bufs=3) as attnp,
            tc.tile_pool(name="ctp", bufs=2) as ctp,
            tc.tile_pool(name="smalls", bufs=2) as smalls,
            tc.tile_pool(name="ostage", bufs=2) as ostage,
            tc.tile_pool(name="mpool", bufs=2) as mpool,
            tc.tile_pool(name="scps", bufs=2, space="PSUM") as scps,
            tc.tile_pool(name="cxps", bufs=2, space="PSUM") as cxps,
        ):
            xpools = {"q": xq_pool, "k": xk_pool, "v": xv_pool}

            # ---------------- persistent tiles + upfront DMAs ------------
            # DMA emission order tracks first use: x(q,0) + w_q first.
            qt_sb = persist.tile([128, 4, S], BF16, tag="qt")
            kt_sb = persist.tile([128, 4, S], BF16, tag="kt")
            v_sb = persist.tile([128, NT, 8, 65], BF16, tag="v")

            x_tiles = {}
            dma_engines = [nc.sync, nc.gpsimd, nc.scalar, nc.gpsimd]

            def fetch_x(s, t, spread=False):
                if (s, t) in x_tiles:
                    return x_tiles[(s, t)]
                xt = xpools[s].tile([128, 8, 512], BF16, tag=f"x{s}")
                for h in range(4):
                    eng = dma_engines[h] if spread else nc.gpsimd
                    eng.dma_start(
                        xt[:, 2 * h:2 * h + 2, :],
                        xT[s][:, t, 2 * h:2 * h + 2, :],
                    )
                x_tiles[(s, t)] = xt
                return xt

            w_sb = {}

            def fetch_w(s):
                w1sb = persist.tile([128, 8, GCH], BF16, tag=f"w1{s}")
                w2sb = persist.tile([128, 8, GCH], BF16, tag=f"w2{s}")
                for wsb, wd in ((w1sb, w1T[s]), (w2sb, w2T[s])):
                    for h in range(4):
                        dma_engines[h].dma_start(
                            wsb[:, 2 * h:2 * h + 2, :],
                            wd[:, 2 * h:2 * h + 2, :])
                w_sb[s] = (w1sb, w2sb)

            fetch_x("q", 0, spread=True)
            fetch_w("q")
            fetch_x("k", 0, spread=True)
            fetch_w("k")
            fetch_x("v", 0, spread=True)
            fetch_w("v")

            bias_sb = {}
            for s in "qk":
                for bn in ("b1", "b2", "b1h"):
                    t_ = persist.tile([128, 4], F32, tag=f"{bn}{s}")
                    nc.sync.dma_start(t_[:], bias_d[f"{bn}_{s}"][:])
                    bias_sb[f"{bn}_{s}"] = t_
            b1vr = persist.tile([1, GCH], BF16, tag="b1v")
            b2vr = persist.tile([1, GCH], BF16, tag="b2v")
            nc.sync.dma_start(b1vr[:], b1v_d[:])
            nc.sync.dma_start(b2vr[:], b2v_d[:])
            woT_sb = persist.tile([128, 4, D], BF16, tag="wo")
            nc.sync.dma_start(woT_sb[:], woT_d[:])

            onesf = persist.tile([1, 128], F32, tag="onesf")
            ones_r = persist.tile([1, 128], BF16, tag="ones_r")
            nc.any.memset(onesf[:], 1.0)
            nc.vector.tensor_copy(ones_r[:], onesf[:])
            onescol = persist.tile([128, 1], F32, tag="onescol")
            nc.any.memset(onescol[:], 1.0)
            nc.vector.tensor_copy(
                v_sb[:, :, :, 64:65],
                onescol[:, None, :].to_broadcast([128, NT, 8, 1]),
            )
            # selector for the denominator broadcast: one K=33 matmul puts
            # rec row 0 on out rows 0..63 and rec row 32 on rows 64..127.
            # (partition bases must be 32-aligned, hence rows 0/32; the
            # pad rows stay memset so 0*pad contributes exact zeros.)
            sel2 = persist.tile([33, 128], BF16, tag="sel2")
            nc.any.memset(sel2[:], 0.0)
            nc.any.memset(sel2[0:1, 0:64], 1.0)
            nc.any.memset(sel2[32:33, 64:128], 1.0)
            den_sb = persist.tile([33, 512], F32, tag="den")
            nc.any.memset(den_sb[:], 1.0)
            rec_sb = persist.tile([33, 512], F32, tag="rec")
            nc.any.memset(rec_sb[:], 1.0)
            rec_bf = persist.tile([33, 512], BF16, tag="recbf")
            nc.any.memset(rec_bf[:], 1.0)

            patc_sb = neye_sb = None
            if mask_mode == "causal":
                patc_sb = persist.tile([128, 128], BF16, tag="patc")
                nc.sync.dma_start(patc_sb[:], patc_d[:])
                neye_sb = persist.tile([128, 128], BF16, tag="neye")
                nc.sync.dma_start(neye_sb[:], neye_d[:])

            # ------------------- projection items ------------------------
            def emit_qk_item(s, t, jh, jj, pool=None):
                """16 MMs + epilogue for channel chunk j=jh*2+jj, seq tile t."""
                j = jh * 2 + jj
                xt = x_tiles[(s, t)]
                w1sb, w2sb = w_sb[s]
                pool = pool if pool is not None else scps
                tg = "sc" if pool is scps else "cx"
                pp = pool.tile([128, 2, 512], F32, tag=tg,
                               name=f"pp{s}{t}{j}")
                ps1, ps2 = pp[:, 0, :], pp[:, 1, :]
                jsl = slice(j * 128, (j + 1) * 128)
                for dc in range(8):
                    nc.tensor.matmul(ps1, w1sb[:, dc, jsl], xt[:, dc, :],
                                     start=(dc == 0), stop=(dc == 7))
                    nc.tensor.matmul(ps2, w2sb[:, dc, jsl], xt[:, dc, :],
                                     start=(dc == 0), stop=(dc == 7))
                act = stage.tile([128, 512], F32, tag="act")
                # act = tanh((ps1+b1)/2)
                nc.scalar.activation(act[:], ps1, ACTF.Tanh, scale=0.5,
                                     bias=bias_sb[f"b1h_{s}"][:, j:j + 1])
                a_sb = stage.tile([128, 512], F32, tag="asb")
                nc.vector.tensor_scalar_add(a_sb[:], ps1,
                                            bias_sb[f"b1_{s}"][:, j:j + 1])
                # act = (1+tanh)*(ps1+b1) = 2*silu(ps1+b1)
                nc.vector.scalar_tensor_tensor(
                    act[:], act[:], 1.0, a_sb[:], op0=ALU.add, op1=ALU.mult)
                dst = (qt_sb if s == "q" else kt_sb)[
                    :, j, t * 512:(t + 1) * 512]
                nc.vector.scalar_tensor_tensor(
                    dst, ps2, bias_sb[f"b2_{s}"][:, j:j + 1], act[:],
                    op0=ALU.add, op1=ALU.mult)

            def emit_v_item(t, j, pool=None):
                """16+2 MMs + epilogue for seq block nt=t*4+j (v path)."""
                xt = x_tiles[("v", t)]
                w1sb, w2sb = w_sb["v"]
                pool = pool if pool is not None else scps
                tg = "sc" if pool is scps else "cx"
                pp = pool.tile([128, 2, 512], F32, tag=tg,
                               name=f"ppv{t}{j}")
                ps1, ps2 = pp[:, 0, :], pp[:, 1, :]
                jsl = slice(j * 128, (j + 1) * 128)
                for dc in range(8):
                    nc.tensor.matmul(ps1, xt[:, dc, jsl], w1sb[:, dc, :],
                                     start=(dc == 0), stop=False)
                    nc.tensor.matmul(ps2, xt[:, dc, jsl], w2sb[:, dc, :],
                                     start=(dc == 0), stop=False)
                nc.tensor.matmul(ps1, ones_r[:], b1vr[:],
                                 start=False, stop=True)
                nc.tensor.matmul(ps2, ones_r[:], b2vr[:],
                                 start=False, stop=True)
                act = stage.tile([128, 512], F32, tag="act")
                nc.scalar.activation(act[:], ps1, ACTF.Tanh, scale=0.5)
                # act = (1+tanh)*ps1 = 2*silu(ps1)
                nc.vector.scalar_tensor_tensor(
                    act[:], act[:], 1.0, ps1, op0=ALU.add, op1=ALU.mult)
                nt_i = t * 4 + j
                nc.vector.tensor_tensor(
                    v_sb[:, nt_i, :, 0:64],
                    ps2.rearrange("p (h d) -> p h d", h=8),
                    act[:].rearrange("p (h d) -> p h d", h=8),
                    ALU.mult)

            # ------------------- attention helpers ------------------------
            def emit_oproj_chunk(qg, ns, oh, ct_qg):
                nt_i = qg * 4 + ns
                nsl = slice(ns * 128, (ns + 1) * 128)
                pof = scps.tile([128, 2, 512], F32, tag="sc",
                                name=f"po{qg}{ns}{oh}")
                po = pof[:, 0, :]
                for j in range(4):
                    nc.tensor.matmul(
                        po, ct_qg[:, j, nsl],
                        woT_sb[:, j, oh * 512:(oh + 1) * 512],
                        start=(j == 0), stop=(j == 3))
                ot = ostage.tile([128, 512], F32, tag="ot")
                nc.vector.tensor_copy(ot[:], po)
                nc.sync.dma_start(
                    pout_d[nt_i * 128:(nt_i + 1) * 128,
                           oh * 512:(oh + 1) * 512],
                    ot[:])

            # ===================== main pipeline ==========================
            # q(0), k(0) projections up front (x DMAs pipelined by pools).
            for s in ("q", "k"):
                for jh in range(2):
                    for jj in range(2):
                        emit_qk_item(s, 0, jh, jj)
            if mask_mode != "causal":
                # attention for every q-group needs ALL of k and v: emit
                # the remaining projections up front, drip only q(t>0).
                for t in range(1, 4):
                    fetch_x("k", t)
                    for jh in range(2):
                        for jj in range(2):
                            emit_qk_item("k", t, jh, jj)
                for t in range(4):
                    fetch_x("v", t)
                    for j in range(4):
                        emit_v_item(t, j)

            pending = []          # deferred projection items (drip queue)
            oproj_pending = []    # deferred oproj chunks of previous qg
            norm_pending = []     # deferred normalization part-B closures

            def drip(n=1, pool=None):
                for _ in range(n):
                    if pending:
                        pending.pop(0)(pool)
                    elif oproj_pending:
                        oproj_pending.pop(0)()

            for qg in range(4):
                kcmax = kc_count(qg)
                qsl0 = qg * 512
                ct_qg = ctp.tile([128, 4, 512], BF16, tag="ct",
                                 name=f"ct{qg}")

                # stock the drip queue with next tile's q/k items; v(qg)
                # items are emitted in-line right before their first use.
                if qg < 3:
                    fetch_x("q", qg + 1)
                    if mask_mode == "causal":
                        fetch_x("k", qg + 1)
                        fetch_x("v", qg + 1)
                    srcs = ("q", "k") if mask_mode == "causal" else ("q",)
                    for s in srcs:
                        for jh in range(2):
                            for jj in range(2):
                                pending.append(
                                    (lambda pool, s=s, t=qg + 1, jh=jh, jj=jj:
                                     emit_qk_item(s, t, jh, jj, pool)))

                mtiles = None
                if mask_mode == "general":
                    mtiles = []
                    mt_sb = mpool.tile([128, NT, 512], BF16, tag="mt")
                    for kc in range(kcmax):
                        nc.sync.dma_start(
                            mt_sb[:, kc, :],
                            m01T_d[kc * 128:(kc + 1) * 128,
                                   qsl0:qsl0 + 512])
                        mtiles.append(mt_sb[:, kc, :])

                units = 0
                for pj in range(4):
                    ctx = cxps.tile([128, 2, 512], F32, tag="cx",
                                    name=f"ctx{qg}{pj}")
                    for kc in range(kcmax):
                        ksl = slice(kc * 128, (kc + 1) * 128)
                        diag_i = kc - 4 * qg
                        c0 = 128 * diag_i if (
                            mask_mode == "causal" and diag_i >= 0) else 0
                        ncols = 512 - c0
                        qca = slice(qsl0 + c0, qsl0 + 512)

                        diag = mask_mode == "causal" and diag_i >= 0
                        sc = scps.tile([128, 2, 512], F32, tag="sc",
                                       name=f"sc{qg}{pj}{kc}")
                        for par in range(2):
                            bp = par * 64
                            tp = (bp, 0) if SCORE_TILE_POS else None
                            nc.tensor.matmul(
                                sc[:, par, c0:],
                                kt_sb[bp:bp + 64, pj, ksl],
                                qt_sb[bp:bp + 64, pj, qca],
                                start=True, stop=not diag,
                                tile_position=tp)
                        if diag:
                            # add -1e9 to the masked (upper) triangle of the
                            # partial block so exp() yields exact zeros.
                            for par in range(2):
                                nc.tensor.matmul(
                                    sc[:, par, c0:c0 + 128],
                                    neye_sb[:], patc_sb[:],
                                    start=False, stop=True)

                        # v(qg) projection items drip in right after the
                        # first pj's scores that need them.
                        if pj == 0 and mask_mode == "causal" and diag_i >= 0:
                            emit_v_item(qg, diag_i, cxps)

                        attn = attnp.tile([128, 2, 512], BF16, tag="at",
                                          name=f"at{qg}{pj}{kc}")
                        nc.scalar.activation(attn[:, :, c0:], sc[:, :, c0:],
                                             ACTF.Exp)
                        if mask_mode == "general":
                            nc.gpsimd.tensor_tensor(
                                attn[:], attn[:],
                                mtiles[kc][:, None, :].to_broadcast(
                                    [128, 2, 512]),
                                ALU.mult)
                        for par in range(2):
                            hl = 2 * pj + par
                            nc.tensor.matmul(
                                ctx[0:65, par, c0:],
                                v_sb[:, kc, hl, :],
                                attn[:, par, c0:],
                                start=(kc == 0),
                                stop=(kc == kcmax - 1))
                        units += 1
                        if kc == 2 and norm_pending:
                            norm_pending.pop(0)()
                        if (units % DRIP_EVERY == 0 and 3 <= kc <= kcmax - 3):
                            drip(pool=cxps)

                    # fill the PE queue before the serial norm chain:
                    # oproj chunks of the previous q-group + one drip item.
                    for _ in range(2):
                        if oproj_pending:
                            oproj_pending.pop(0)()
                    if pending:
                        pending.pop(0)(scps)

                    # ---- normalization, part A: DVE reciprocal chain ----
                    nc.vector.tensor_copy(den_sb[0:1, :], ctx[64:65, 0, :])
                    nc.vector.tensor_copy(den_sb[32:33, :], ctx[64:65, 1, :])
                    nc.vector.reciprocal_approx_fast(rec_sb[:], den_sb[:])
                    nc.vector.tensor_copy(rec_bf[:], rec_sb[:])

                    # part B (bc matmul + ct multiplies) is deferred into the
                    # next head pair's kc loop so the PE FIFO never waits on
                    # the DVE chain above.
                    def emit_normB(qg=qg, pj=pj, ctx=ctx, ct_qg=ct_qg):
                        bc_full = scps.tile([128, 2, 512], F32, tag="sc",
                                            name=f"bc{qg}{pj}")
                        bc_ps = bc_full[:, 0, :]
                        nc.tensor.matmul(bc_ps, sel2[:], rec_bf[:])
                        bc_sb = smalls.tile([128, 512], BF16, tag="bcs")
                        nc.vector.tensor_copy(bc_sb[:], bc_ps)
                        for par in range(2):
                            bp = par * 64
                            nc.vector.tensor_tensor(
                                ct_qg[bp:bp + 64, pj, :],
                                ctx[0:64, par, :],
                                bc_sb[bp:bp + 64, :], ALU.mult)
                    norm_pending.append(emit_normB)

                # flush remaining projection items before the next q-group.
                # normB of the last head pair stays deferred into the next
                # q-group's first kc loop (it would block the PE FIFO here).
                while pending:
                    drip()
                if qg == 3:
                    while norm_pending:
                        norm_pending.pop(0)()
                while oproj_pending:
                    oproj_pending.pop(0)()
                for ns in range(4):
                    for oh in range(2):
                        oproj_pending.append(
                            (lambda qg=qg, ns=ns, oh=oh, ct=ct_qg:
                             emit_oproj_chunk(qg, ns, oh, ct)))

            while oproj_pending:
                oproj_pending.pop(0)()
    nc.compile()
    return nc


def _host_prepare(inputs):
    """Split the full problem into 8 per-core input maps + host-side info."""
    q = np.asarray(inputs["query"], dtype=np.float32)
    k = np.asarray(inputs["key"], dtype=np.float32)
    v = np.asarray(inputs["value"], dtype=np.float32)
    mask = np.asarray(inputs["mask"])
    w = {n: np.asarray(inputs[n], dtype=np.float32)
         for n in ("wq1", "wq2", "wk1", "wk2", "wv1", "wv2", "wo")}
    bias = {n: np.asarray(inputs[n], dtype=np.float32)
            for n in ("bq1", "bq2", "bk1", "bk2", "bv1", "bv2", "bo")}

    m = mask.reshape(S, S)
    if np.array_equal(m != 0, np.tril(np.ones((S, S), bool))):
        mask_mode = "causal"
    elif np.all(m != 0):
        mask_mode = "full"
    else:
        mask_mode = "general"

    patc = None
    m01T = None
    if mask_mode == "causal":
        kk = np.arange(128)[:, None]
        qq = np.arange(128)[None, :]
        patc = np.ascontiguousarray((kk > qq).astype(np.float32))
    elif mask_mode == "general":
        m01T = np.ascontiguousarray((m != 0).T.astype(np.float32))

    scale = 1.0 / np.sqrt(DK).astype(np.float32)

    import ml_dtypes

    def cvt(a):
        return np.ascontiguousarray(a).astype(ml_dtypes.bfloat16)

    in_maps = []
    for c in range(NCORES):
        b, g = divmod(c, 2)
        sl = slice(g * GCH, (g + 1) * GCH)
        def xprep(a):
            # [D, S] -> [128 p, 4 t, 8 dc, 512 o]; x[dc*128+p, t*512+o]
            return cvt(a.T.reshape(8, 128, 4, 512).transpose(1, 2, 0, 3))

        def wprep(a):
            # [D, GCH] -> [128 p, 8 dc, 512 o]; w[dc*128+p, o]
            return cvt(a.reshape(8, 128, GCH).transpose(1, 0, 2))

        im = {
            "xqT": xprep(q[b]),
            "xkT": xprep(k[b]),
            "xvT": xprep(v[b]),
            "w1T_q": wprep(w["wq1"][sl].T),
            # fold the 1/sqrt(dk) score scale into the non-silu Q branch,
            # and 0.5 everywhere (silu computed as A*(1+tanh(A/2)) = 2*silu)
            "w2T_q": wprep(w["wq2"][sl].T * (scale * 0.5)),
            "w2T_k": wprep(w["wk2"][sl].T * 0.5),
            "w2T_v": wprep(w["wv2"][sl].T * 0.5),
            "w1T_k": wprep(w["wk1"][sl].T),
            "w1T_v": wprep(w["wv1"][sl].T),
            "b1_q": np.ascontiguousarray(bias["bq1"][sl].reshape(4, 128).T),
            "b1h_q": np.ascontiguousarray(
                (bias["bq1"][sl] * 0.5).reshape(4, 128).T),
            "b2_q": np.ascontiguousarray(
                (bias["bq2"][sl] * (scale * 0.5)).reshape(4, 128).T),
            "b1_k": np.ascontiguousarray(bias["bk1"][sl].reshape(4, 128).T),
            "b1h_k": np.ascontiguousarray(
                (bias["bk1"][sl] * 0.5).reshape(4, 128).T),
            "b2_k": np.ascontiguousarray(
                (bias["bk2"][sl] * 0.5).reshape(4, 128).T),
            "b1_v": cvt(bias["bv1"][sl].reshape(1, GCH)),
            "b2_v": cvt((bias["bv2"][sl] * 0.5).reshape(1, GCH)),
            "woT": cvt(
                w["wo"][:, sl].T.reshape(4, 128, D).transpose(1, 0, 2)),
        }
        if mask_mode == "causal":
            im["patc"] = cvt(patc)
            im["neye"] = cvt(-1e9 * np.eye(128, dtype=np.float32))
        elif mask_mode == "general":
            im["m01T"] = cvt(m01T)
        in_maps.append(im)
    return mask_mode, in_maps, bias["bo"]


def kernel(**inputs):
    global LAST_RESULT
    mask_mode, in_maps, bo = _host_prepare(inputs)
    nc = build_program(mask_mode)

    import concourse.bass_utils as bu

    if TRACE:
        import types

        try:
            from trn_agent_boot.trn_boot import _ntff_profile_via_ctypes

            hook = _ntff_profile_via_ctypes("/opt/axon/libaxon_pjrt.so")
            m = types.ModuleType("antenv.axon_hooks")
            m.get_axon_ntff_profile_hook = lambda: hook
            import antenv  # noqa: F401

            sys.modules["antenv.axon_hooks"] = m
            bu.upload_artifacts = lambda d: "local://skipped"
        except Exception as e:
            print("profiling hook install failed:", e)

    res = bu.run_bass_kernel_spmd(
        nc, in_maps, core_ids=list(range(NCORES)),
        trace=TRACE, trace_cores=TRACE_CORES,
    )
    LAST_RESULT = res

    out = np.empty((B, S, D), dtype=np.float32)
    for b in range(B):
        out[b] = (res.results[2 * b]["pout"] + res.results[2 * b + 1]["pout"]
                  + bo[None, :])
    return out


# revision 23
# speedup vs baseline: 1.2423x; 1.0638x over previous
"""SwiGLU-projected causal MHA (B=4, S=2048, D=1024, H=16) on 8 TRN2 NeuronCores.

Sharding: core c -> (batch b = c//2, head-group g = c%2).  Each core computes
the SwiGLU Q/K/V projections for its 512 output channels (= 8 heads) of its
batch, runs causal attention for those heads, and produces a partial output
projection (contraction over its 512 channels).  The host sums the two
partials per batch and adds the output bias.

v2: software-pipelined emission.  Projections are split into fine-grained
items; attention for q-group t starts right after q(t)/k(t) are projected,
and the remaining projection items (v(t), q(t+1), k(t+1)) are dripped into
the attention kc loops to fill the PE idle created by the ACT-bound exp
stream.  Score matmul pairs use explicit row-group tile_position for PE
concurrency; exp runs once per kc over both heads of a pair via a strided
2-bank PSUM read; diagonal blocks are column-truncated (causal); the partial
mask multiply runs on the Pool engine; softmax normalization uses
reciprocal_approx_fast plus a single selector matmul to broadcast both
reciprocal rows in one shot.
"""
import sys

sys.path.insert(0, "/opt/trn_rl_repo")
import numpy as np

import concourse.bacc as bacc
import concourse.tile as tile
import concourse.mybir as mybir

B, S, D = 4, 2048, 1024
H, DK = 16, 64
NCORES = 8
GCH = 512          # channels per core (8 heads)
NT = S // 128      # 16 seq chunks
F32 = mybir.dt.float32
F32R = mybir.dt.float32r
BF16 = mybir.dt.bfloat16
ACTF = mybir.ActivationFunctionType
ALU = mybir.AluOpType

TRACE = False          # set by test.py for profiling runs
TRACE_CORES = None
LAST_RESULT = None     # BassKernelResults stash for test.py

DRIP_EVERY = 5         # attention kc-units between dripped projection items
SCORE_TILE_POS = True  # explicit row-group tile_position on score MM pairs


def build_program(mask_mode):
    """mask_mode: 'causal' (tril), 'full' (all ones), 'general' (arbitrary)."""
    nc = bacc.Bacc("TRN2", target_bir_lowering=False, debug=False)

    xT = {s: nc.dram_tensor(f"x{s}T", [128, 4, 8, 512], BF16,
                            kind="ExternalInput")
          for s in "qkv"}
    w1T = {s: nc.dram_tensor(f"w1T_{s}", [128, 8, GCH], BF16,
                             kind="ExternalInput")
           for s in "qkv"}
    w2T = {s: nc.dram_tensor(f"w2T_{s}", [128, 8, GCH], BF16,
                             kind="ExternalInput")
           for s in "qkv"}
    bias_d = {}
    for s in "qk":
        for bn in ("b1", "b2", "b1h"):
            bias_d[f"{bn}_{s}"] = nc.dram_tensor(f"{bn}_{s}", [128, 4], F32,
                                                 kind="ExternalInput")
    b1v_d = nc.dram_tensor("b1_v", [1, GCH], BF16, kind="ExternalInput")
    b2v_d = nc.dram_tensor("b2_v", [1, GCH], BF16, kind="ExternalInput")
    woT_d = nc.dram_tensor("woT", [128, 4, D], BF16, kind="ExternalInput")
    patc_d = neye_d = m01T_d = None
    if mask_mode == "causal":
        patc_d = nc.dram_tensor("patc", [128, 128], BF16, kind="ExternalInput")
        neye_d = nc.dram_tensor("neye", [128, 128], BF16, kind="ExternalInput")
    elif mask_mode == "general":
        m01T_d = nc.dram_tensor("m01T", [S, S], BF16, kind="ExternalInput")
    pout_d = nc.dram_tensor("pout", [S, D], F32, kind="ExternalOutput")

    def kc_count(qg):
        return 4 * qg + 4 if mask_mode == "causal" else NT

    with tile.TileContext(nc) as tc:
        with (
            tc.tile_pool(name="persist", bufs=1) as persist,
            tc.tile_pool(name="xq", bufs=2) as xq_pool,
            tc.tile_pool(name="xk", bufs=2) as xk_pool,
            tc.tile_pool(name="xv", bufs=2) as xv_pool,
            tc.tile_pool(name="stage", bufs=3) as stage,
            tc.tile_pool(name="attnp", bufs=3) as attnp,
            tc.tile_pool(name="ctp", bufs=2) as ctp,
            tc.tile_pool(name="smalls", bufs=2) as smalls,
            tc.tile_pool(name="ostage", bufs=2) as ostage,
            tc.tile_pool(name="mpool", bufs=2) as mpool,
            tc.tile_pool(name="scps", bufs=2, space="PSUM") as scps,
            tc.tile_pool(name="cxps", bufs=2, space="PSUM") as cxps,
        ):
            xpools = {"q": xq_pool, "k": xk_pool, "v": xv_pool}

            # ---------------- persistent tiles + upfront DMAs ------------
            # DMA emission order tracks first use: x(q,0) + w_q first.
            qt_sb = persist.tile([128, 4, S], BF16, tag="qt")
            kt_sb = persist.tile([128, 4, S], BF16, tag="kt")
            v_sb = persist.tile([128, NT, 8, 65], BF16, tag="v")

            x_tiles = {}
            dma_engines = [nc.sync, nc.gpsimd, nc.scalar, nc.gpsimd]

            def fetch_x(s, t, spread=False):
                if (s, t) in x_tiles:
                    return x_tiles[(s, t)]
                xt = xpools[s].tile([128, 8, 512], BF16, tag=f"x{s}")
                for h in range(4):
                    eng = dma_engines[h] if spread else nc.gpsimd
                    eng.dma_start(
                        xt[:, 2 * h:2 * h + 2, :],
                        xT[s][:, t, 2 * h:2 * h + 2, :],
                    )
                x_tiles[(s, t)] = xt
                return xt

            w_sb = {}

            def fetch_w(s):
                w1sb = persist.tile([128, 8, GCH], BF16, tag=f"w1{s}")
                w2sb = persist.tile([128, 8, GCH], BF16, tag=f"w2{s}")
                for wsb, wd in ((w1sb, w1T[s]), (w2sb, w2T[s])):
                    for h in range(4):
                        dma_engines[h].dma_start(
                            wsb[:, 2 * h:2 * h + 2, :],
                            wd[:, 2 * h:2 * h + 2, :])
                w_sb[s] = (w1sb, w2sb)

            fetch_x("q", 0, spread=True)
            fetch_w("q")
            fetch_x("k", 0, spread=True)
            fetch_w("k")
            fetch_x("v", 0, spread=True)
            fetch_w("v")

            bias_sb = {}
            for s in "qk":
                for bn in ("b1", "b2", "b1h"):
                    t_ = persist.tile([128, 4], F32, tag=f"{bn}{s}")
                    nc.sync.dma_start(t_[:], bias_d[f"{bn}_{s}"][:])
                    bias_sb[f"{bn}_{s}"] = t_
            b1vr = persist.tile([1, GCH], BF16, tag="b1v")
            b2vr = persist.tile([1, GCH], BF16, tag="b2v")
            nc.sync.dma_start(b1vr[:], b1v_d[:])
            nc.sync.dma_start(b2vr[:], b2v_d[:])
            woT_sb = persist.tile([128, 4, D], BF16, tag="wo")
            nc.sync.dma_start(woT_sb[:], woT_d[:])

            onesf = persist.tile([1, 128], F32, tag="onesf")
            ones_r = persist.tile([1, 128], BF16, tag="ones_r")
            nc.any.memset(onesf[:], 1.0)
            nc.vector.tensor_copy(ones_r[:], onesf[:])
            onescol = persist.tile([128, 1], F32, tag="onescol")
            nc.any.memset(onescol[:], 1.0)
            nc.vector.tensor_copy(
                v_sb[:, :, :, 64:65],
                onescol[:, None, :].to_broadcast([128, NT, 8, 1]),
            )
            # selector for the denominator broadcast: one K=33 matmul puts
            # rec row 0 on out rows 0..63 and rec row 32 on rows 64..127.
            # (partition bases must be 32-aligned, hence rows 0/32; the
            # pad rows stay memset so 0*pad contributes exact zeros.)
            sel2 = persist.tile([33, 128], BF16, tag="sel2")
            nc.any.memset(sel2[:], 0.0)
            nc.any.memset(sel2[0:1, 0:64], 1.0)
            nc.any.memset(sel2[32:33, 64:128], 1.0)
            den_sb = persist.tile([33, 512], F32, tag="den")
            nc.any.memset(den_sb[:], 1.0)
            rec_sb = persist.tile([33, 512], F32, tag="rec")
            nc.any.memset(rec_sb[:], 1.0)
            rec_bf = persist.tile([33, 512], BF16, tag="recbf")
            nc.any.memset(rec_bf[:], 1.0)

            patc_sb = neye_sb = None
            if mask_mode == "causal":
                patc_sb = persist.tile([128, 128], BF16, tag="patc")
                nc.sync.dma_start(patc_sb[:], patc_d[:])
                neye_sb = persist.tile([128, 128], BF16, tag="neye")
                nc.sync.dma_start(neye_sb[:], neye_d[:])

            # ------------------- projection items ------------------------
            def emit_qk_item(s, t, jh, jj, pool=None):
                """16 MMs + epilogue for channel chunk j=jh*2+jj, seq tile t."""
                j = jh * 2 + jj
                xt = x_tiles[(s, t)]
                w1sb, w2sb = w_sb[s]
                pool = pool if pool is not None else scps
                tg = "sc" if pool is scps else "cx"
                pp = pool.tile([128, 2, 512], F32, tag=tg,
                               name=f"pp{s}{t}{j}")
                ps1, ps2 = pp[:, 0, :], pp[:, 1, :]
                jsl = slice(j * 128, (j + 1) * 128)
                for dc in range(8):
                    nc.tensor.matmul(ps1, w1sb[:, dc, jsl], xt[:, dc, :],
                                     start=(dc == 0), stop=(dc == 7))
                    nc.tensor.matmul(ps2, w2sb[:, dc, jsl], xt[:, dc, :],
                                     start=(dc == 0), stop=(dc == 7))
                act = stage.tile([128, 512], F32, tag="act")
                # act = tanh((ps1+b1)/2)
                nc.scalar.activation(act[:], ps1, ACTF.Tanh, scale=0.5,
                                     bias=bias_sb[f"b1h_{s}"][:, j:j + 1])
                a_sb = stage.tile([128, 512], F32, tag="asb")
                nc.vector.tensor_scalar_add(a_sb[:], ps1,
                                            bias_sb[f"b1_{s}"][:, j:j + 1])
                # act = (1+tanh)*(ps1+b1) = 2*silu(ps1+b1)
                nc.vector.scalar_tensor_tensor(
                    act[:], act[:], 1.0, a_sb[:], op0=ALU.add, op1=ALU.mult)
                dst = (qt_sb if s == "q" else kt_sb)[
                    :, j, t * 512:(t + 1) * 512]
                nc.vector.scalar_tensor_tensor(
                    dst, ps2, bias_sb[f"b2_{s}"][:, j:j + 1], act[:],
                    op0=ALU.add, op1=ALU.mult)

            def emit_v_item(t, j, pool=None):
                """16+2 MMs + epilogue for seq block nt=t*4+j (v path)."""
                xt = x_tiles[("v", t)]
                w1sb, w2sb = w_sb["v"]
                pool = pool if pool is not None else scps
                tg = "sc" if pool is scps else "cx"
                pp = pool.tile([128, 2, 512], F32, tag=tg,
                               name=f"ppv{t}{j}")
                ps1, ps2 = pp[:, 0, :], pp[:, 1, :]
                jsl = slice(j * 128, (j + 1) * 128)
                for dc in range(8):
                    nc.tensor.matmul(ps1, xt[:, dc, jsl], w1sb[:, dc, :],
                                     start=(dc == 0), stop=False)
                    nc.tensor.matmul(ps2, xt[:, dc, jsl], w2sb[:, dc, :],
                                     start=(dc == 0), stop=False)
                nc.tensor.matmul(ps1, ones_r[:], b1vr[:],
                                 start=False, stop=True)
                nc.tensor.matmul(ps2, ones_r[:], b2vr[:],
                                 start=False, stop=True)
                act = stage.tile([128, 512], F32, tag="act")
                nc.scalar.activation(act[:], ps1, ACTF.Tanh, scale=0.5)
                # act = (1+tanh)*ps1 = 2*silu(ps1)
                nc.vector.scalar_tensor_tensor(
                    act[:], act[:], 1.0, ps1, op0=ALU.add, op1=ALU.mult)
                nt_i = t * 4 + j
                nc.vector.tensor_tensor(
                    v_sb[:, nt_i, :, 0:64],
                    ps2.rearrange("p (h d) -> p h d", h=8),
                    act[:].rearrange("p (h d) -> p h d", h=8),
                    ALU.mult)

            # ------------------- attention helpers ------------------------
            def emit_oproj_chunk(qg, ns, oh, ct_qg):
                nt_i = qg * 4 + ns
                nsl = slice(ns * 128, (ns + 1) * 128)
                pof = scps.tile([128, 2, 512], F32, tag="sc",
                                name=f"po{qg}{ns}{oh}")
                po = pof[:, 0, :]
                for j in range(4):
                    nc.tensor.matmul(
                        po, ct_qg[:, j, nsl],
                        woT_sb[:, j, oh * 512:(oh + 1) * 512],
                        start=(j == 0), stop=(j == 3))
                ot = ostage.tile([128, 512], F32, tag="ot")
                nc.vector.tensor_copy(ot[:], po)
                nc.sync.dma_start(
                    pout_d[nt_i * 128:(nt_i + 1) * 128,
                           oh * 512:(oh + 1) * 512],
                    ot[:])

            # ===================== main pipeline ==========================
            # q(0), k(0) projections up front (x DMAs pipelined by pools).
            for s in ("q", "k"):
                for jh in range(2):
                    for jj in range(2):
                        emit_qk_item(s, 0, jh, jj)
            if mask_mode != "causal":
                # attention for every q-group needs ALL of k and v: emit
                # the remaining projections up front, drip only q(t>0).
                for t in range(1, 4):
                    fetch_x("k", t)
                    for jh in range(2):
                        for jj in range(2):
                            emit_qk_item("k", t, jh, jj)
                for t in range(4):
                    fetch_x("v", t)
                    for j in range(4):
                        emit_v_item(t, j)

            pending = []          # deferred projection items (drip queue)
            oproj_pending = []    # deferred oproj chunks of previous qg
            norm_pending = []     # deferred normalization part-B closures

            def drip(n=1, pool=None):
                for _ in range(n):
                    if pending:
                        pending.pop(0)(pool)
                    elif oproj_pending:
                        oproj_pending.pop(0)()

            for qg in range(4):
                kcmax = kc_count(qg)
                qsl0 = qg * 512
                ct_qg = ctp.tile([128, 4, 512], BF16, tag="ct",
                                 name=f"ct{qg}")

                # stock the drip queue with next tile's q/k items; v(qg)
                # items are emitted in-line right before their first use.
                if qg < 3:
                    fetch_x("q", qg + 1)
                    if mask_mode == "causal":
                        fetch_x("k", qg + 1)
                        fetch_x("v", qg + 1)
                    srcs = ("q", "k") if mask_mode == "causal" else ("q",)
                    for s in srcs:
                        for jh in range(2):
                            for jj in range(2):
                                pending.append(
                                    (lambda pool, s=s, t=qg + 1, jh=jh, jj=jj:
                                     emit_qk_item(s, t, jh, jj, pool)))

                mtiles = None
                if mask_mode == "general":
                    mtiles = []
                    mt_sb = mpool.tile([128, NT, 512], BF16, tag="mt")
                    for kc in range(kcmax):
                        nc.sync.dma_start(
                            mt_sb[:, kc, :],
                            m01T_d[kc * 128:(kc + 1) * 128,
                                   qsl0:qsl0 + 512])
                        mtiles.append(mt_sb[:, kc, :])

                units = 0
                ctx_map = {}
                ustate = {}

                def unit_front(pj, kc):
                    ksl = slice(kc * 128, (kc + 1) * 128)
                    diag_i = kc - 4 * qg
                    diag = mask_mode == "causal" and diag_i >= 0
                    c0 = 128 * diag_i if diag else 0
                    qca = slice(qsl0 + c0, qsl0 + 512)
                    sc = scps.tile([128, 2, 512], F32, tag="sc",
                                   name=f"sc{qg}{pj}{kc}")
                    for par in range(2):
                        bp = par * 64
                        tp = (bp, 0) if SCORE_TILE_POS else None
                        nc.tensor.matmul(
                            sc[:, par, c0:],
                            kt_sb[bp:bp + 64, pj, ksl],
                            qt_sb[bp:bp + 64, pj, qca],
                            start=True, stop=not diag,
                            tile_position=tp)
                    if diag:
                        # add -1e9 to the masked (upper) triangle of the
                        # partial block so exp() yields exact zeros.
                        for par in range(2):
                            nc.tensor.matmul(
                                sc[:, par, c0:c0 + 128],
                                neye_sb[:], patc_sb[:],
                                start=False, stop=True)
                    # v(qg) projection items drip in right after the
                    # first pj's scores that need them.
                    if pj == 0 and diag:
                        emit_v_item(qg, diag_i, cxps)
                    attn = attnp.tile([128, 2, 512], BF16, tag="at",
                                      name=f"at{qg}{pj}{kc}")
                    nc.scalar.activation(attn[:, :, c0:], sc[:, :, c0:],
                                         ACTF.Exp)
                    if mask_mode == "general":
                        nc.gpsimd.tensor_tensor(
                            attn[:], attn[:],
                            mtiles[kc][:, None, :].to_broadcast(
                                [128, 2, 512]),
                            ALU.mult)
                    ustate[(pj, kc)] = (attn, c0)

                def unit_back(pj, kc):
                    nonlocal units
                    attn, c0 = ustate.pop((pj, kc))
                    if pj not in ctx_map:
                        ctx_map[pj] = cxps.tile([128, 2, 512], F32, tag="cx",
                                                name=f"ctx{qg}{pj}")
                    ctx = ctx_map[pj]
                    for par in range(2):
                        hl = 2 * pj + par
                        nc.tensor.matmul(
                            ctx[0:65, par, c0:],
                            v_sb[:, kc, hl, :],
                            attn[:, par, c0:],
                            start=(kc == 0),
                            stop=(kc == kcmax - 1))
                    units += 1
                    if kc == 2 and norm_pending:
                        norm_pending.pop(0)()
                    if (units % DRIP_EVERY == 0 and 3 <= kc <= kcmax - 3):
                        drip(pool=cxps)
                    if kc != kcmax - 1:
                        return
                    # ---- last unit of this head pair: boundary work ----
                    for _ in range(2):
                        if oproj_pending:
                            oproj_pending.pop(0)()

                    # normalization, part A: DVE reciprocal chain
                    nc.vector.tensor_copy(den_sb[0:1, :], ctx[64:65, 0, :])
                    nc.vector.tensor_copy(den_sb[32:33, :], ctx[64:65, 1, :])
                    nc.vector.reciprocal_approx_fast(rec_sb[:], den_sb[:])
                    nc.vector.tensor_copy(rec_bf[:], rec_sb[:])

                    # part B (bc matmul + ct multiplies) is deferred into the
                    # next head pair's kc loop so the PE FIFO never waits on
                    # the DVE chain above.
                    def emit_normB(qg=qg, pj=pj, ctx=ctx, ct_qg=ct_qg):
                        bc_full = scps.tile([128, 2, 512], F32, tag="sc",
                                            name=f"bc{qg}{pj}")
                        bc_ps = bc_full[:, 0, :]
                        nc.tensor.matmul(bc_ps, sel2[:], rec_bf[:])
                        bc_sb = smalls.tile([128, 512], BF16, tag="bcs")
                        nc.vector.tensor_copy(bc_sb[:], bc_ps)
                        for par in range(2):
                            bp = par * 64
                            nc.vector.tensor_tensor(
                                ct_qg[bp:bp + 64, pj, :],
                                ctx[0:64, par, :],
                                bc_sb[bp:bp + 64, :], ALU.mult)
                    norm_pending.append(emit_normB)

                # 1-unit software pipeline across the whole q-group: unit
                # u+1's scores+exp are issued before unit u's AVs, so the AV
                # exp-dependency is always already in flight (no per-pj
                # pipeline refill).
                seq = [(pj, kc) for pj in range(4) for kc in range(kcmax)]
                unit_front(*seq[0])
                for idx in range(len(seq)):
                    if idx + 1 < len(seq):
                        unit_front(*seq[idx + 1])
                    unit_back(*seq[idx])

                # flush remaining projection items before the next q-group.
                # normB of the last head pair stays deferred into the next
                # q-group's first kc loop (it would block the PE FIFO here).
                while pending:
                    drip()
                if qg == 3:
                    while norm_pending:
                        norm_pending.pop(0)()
                while oproj_pending:
                    oproj_pending.pop(0)()
                for ns in range(4):
                    for oh in range(2):
                        oproj_pending.append(
                            (lambda qg=qg, ns=ns, oh=oh, ct=ct_qg:
                             emit_oproj_chunk(qg, ns, oh, ct)))

            while oproj_pending:
                oproj_pending.pop(0)()
    nc.compile()
    return nc


def _host_prepare(inputs):
    """Split the full problem into 8 per-core input maps + host-side info."""
    q = np.asarray(inputs["query"], dtype=np.float32)
    k = np.asarray(inputs["key"], dtype=np.float32)
    v = np.asarray(inputs["value"], dtype=np.float32)
    mask = np.asarray(inputs["mask"])
    w = {n: np.asarray(inputs[n], dtype=np.float32)
         for n in ("wq1", "wq2", "wk1", "wk2", "wv1", "wv2", "wo")}
    bias = {n: np.asarray(inputs[n], dtype=np.float32)
            for n in ("bq1", "bq2", "bk1", "bk2", "bv1", "bv2", "bo")}

    m = mask.reshape(S, S)
    if np.array_equal(m != 0, np.tril(np.ones((S, S), bool))):
        mask_mode = "causal"
    elif np.all(m != 0):
        mask_mode = "full"
    else:
        mask_mode = "general"

    patc = None
    m01T = None
    if mask_mode == "causal":
        kk = np.arange(128)[:, None]
        qq = np.arange(128)[None, :]
        patc = np.ascontiguousarray((kk > qq).astype(np.float32))
    elif mask_mode == "general":
        m01T = np.ascontiguousarray((m != 0).T.astype(np.float32))

    scale = 1.0 / np.sqrt(DK).astype(np.float32)

    import ml_dtypes

    def cvt(a):
        return np.ascontiguousarray(a).astype(ml_dtypes.bfloat16)

    in_maps = []
    for c in range(NCORES):
        b, g = divmod(c, 2)
        sl = slice(g * GCH, (g + 1) * GCH)
        def xprep(a):
            # [D, S] -> [128 p, 4 t, 8 dc, 512 o]; x[dc*128+p, t*512+o]
            return cvt(a.T.reshape(8, 128, 4, 512).transpose(1, 2, 0, 3))

        def wprep(a):
            # [D, GCH] -> [128 p, 8 dc, 512 o]; w[dc*128+p, o]
            return cvt(a.reshape(8, 128, GCH).transpose(1, 0, 2))

        im = {
            "xqT": xprep(q[b]),
            "xkT": xprep(k[b]),
            "xvT": xprep(v[b]),
            "w1T_q": wprep(w["wq1"][sl].T),
            # fold the 1/sqrt(dk) score scale into the non-silu Q branch,
            # and 0.5 everywhere (silu computed as A*(1+tanh(A/2)) = 2*silu)
            "w2T_q": wprep(w["wq2"][sl].T * (scale * 0.5)),
            "w2T_k": wprep(w["wk2"][sl].T * 0.5),
            "w2T_v": wprep(w["wv2"][sl].T * 0.5),
            "w1T_k": wprep(w["wk1"][sl].T),
            "w1T_v": wprep(w["wv1"][sl].T),
            "b1_q": np.ascontiguousarray(bias["bq1"][sl].reshape(4, 128).T),
            "b1h_q": np.ascontiguousarray(
                (bias["bq1"][sl] * 0.5).reshape(4, 128).T),
            "b2_q": np.ascontiguousarray(
                (bias["bq2"][sl] * (scale * 0.5)).reshape(4, 128).T),
            "b1_k": np.ascontiguousarray(bias["bk1"][sl].reshape(4, 128).T),
            "b1h_k": np.ascontiguousarray(
                (bias["bk1"][sl] * 0.5).reshape(4, 128).T),
            "b2_k": np.ascontiguousarray(
                (bias["bk2"][sl] * 0.5).reshape(4, 128).T),
            "b1_v": cvt(bias["bv1"][sl].reshape(1, GCH)),
            "b2_v": cvt((bias["bv2"][sl] * 0.5).reshape(1, GCH)),
            "woT": cvt(
                w["wo"][:, sl].T.reshape(4, 128, D).transpose(1, 0, 2)),
        }
        if mask_mode == "causal":
            im["patc"] = cvt(patc)
            im["neye"] = cvt(-1e9 * np.eye(128, dtype=np.float32))
        elif mask_mode == "general":
            im["m01T"] = cvt(m01T)
        in_maps.append(im)
    return mask_mode, in_maps, bias["bo"]


def kernel(**inputs):
    global LAST_RESULT
    mask_mode, in_maps, bo = _host_prepare(inputs)
    nc = build_program(mask_mode)

    import concourse.bass_utils as bu

    if TRACE:
        import types

        try:
            from trn_agent_boot.trn_boot import _ntff_profile_via_ctypes

            hook = _ntff_profile_via_ctypes("/opt/axon/libaxon_pjrt.so")
            m = types.ModuleType("antenv.axon_hooks")
            m.get_axon_ntff_profile_hook = lambda: hook
            import antenv  # noqa: F401

            sys.modules["antenv.axon_hooks"] = m
            bu.upload_artifacts = lambda d: "local://skipped"
        except Exception as e:
            print("profiling hook install failed:", e)

    res = bu.run_bass_kernel_spmd(
        nc, in_maps, core_ids=list(range(NCORES)),
        trace=TRACE, trace_cores=TRACE_CORES,
    )
    LAST_RESULT = res

    out = np.empty((B, S, D), dtype=np.float32)
    for b in range(B):
        out[b] = (res.results[2 * b]["pout"] + res.results[2 * b + 1]["pout"]
                  + bo[None, :])
    return out
